# revision 14
# baseline (speedup 1.0000x reference)
"""Binarized-MLP (BinaryNet) forward on 8 Trainium2 NeuronCores.

Reference computation (per nn_FC_large):
    h = sign(x[:, :768]) @ sign(w1).T + b1 ; BN1 ; -> sign
    h = sign(h) @ sign(w2).T + b2         ; BN2 ; -> sign
    h = sign(h) @ sign(w3).T + b3         ; BN3 ; -> sign
    h = sign(h) @ sign(w4).T + b4         ; BN4 ; log_softmax

Strategy (data parallel, batch 16384 -> 2048 rows/core):
  * Activations are a in {0,1} fp8 (a = [pre-act >= 0]); the identity
    sign_mm = 2*(Wsign @ a) - rowsum(Wsign) folds the rowsum into
    per-neuron integer thresholds, so every epilogue is one DVE op.
  * Matmuls run in fp8e4 with perf_mode=DoubleRow (K=256/instruction).
  * Layers 2+3 (4096x4096) use one level of Strassen-Winograd: 7 products
    of half-size blocks instead of 8 (PE work x7/8). Weight-side combos
    (host, values in {-8..8}) and activation-side combos T1..T4 (GpSimd,
    values in {-2..2}) are exact in fp8; products are integers well inside
    fp32, so the whole pipeline stays bit-exact w.r.t. the reference
    (weights carry a 2x so thresholds are simply doubled).
    The 7 output-side adds fold thresholds + compare into DVE
    scalar_tensor_tensor ops: e.g. C11 = [(P1 - t) >= (-P2)] with the
    (-P2) product negated host-side.
  * Layer-4 logits are PE-transposed to batch-major; log_softmax runs
    on-device (DVE/ACT) with Exp batched to avoid ACT table thrash.

Everything is hardcoded for x:[16384,784], layers 768->4096->4096->4096->10.
"""

import numpy as np
import ml_dtypes
from contextlib import ExitStack

import concourse.mybir as mybir
import concourse.tile as tile
from concourse import bacc
from concourse.bass_utils import run_bass_kernel_spmd
from concourse.masks import make_identity

FP32 = mybir.dt.float32
BF16 = mybir.dt.bfloat16
FP8 = mybir.dt.float8e4
NP_FP8 = ml_dtypes.float8_e4m3
NP_BF16 = ml_dtypes.bfloat16

EPS = 1e-5
B, IND, HID, OUT = 16384, 768, 4096, 10
N_CORES = 8
BC = B // N_CORES  # 2048 batch rows per core

# Knobs (test.py may flip TRACE before calling kernel()).
TRACE = False
TRACE_KWARGS = {}
LAST_RESULTS = None  # BassKernelResults of the most recent run

ALU = mybir.AluOpType


# --------------------------------------------------------------------------
# Device program
# --------------------------------------------------------------------------

def _strassen_layer(nc, wpool, tpool, spool, psum_pool, act_in, wS, thr_sb,
                    act_out, bc):
    """One 4096x4096 binarized layer via 1-level Strassen-Winograd.

    act_in : SBUF AP [128, 16, 2, bc] fp8 {0,1}
    wS     : DRAM [16, 128, 7, 8, 2, 128] fp8 (7 left-combos, DR layout)
    thr_sb : SBUF [128, 32] fp32 (2x integer-snapped thresholds)
    act_out: SBUF AP [128, 16, 2, bc] fp8
    """
    DR = mybir.MatmulPerfMode.DoubleRow
    for blk in range(2):
        nl = slice(512 * blk, 512 * blk + 512)
        nr = slice(1024 + 512 * blk, 1536 + 512 * blk)
        # activation-side combos for this block (values in {-2..2}, exact)
        t1 = tpool.tile([128, 8, 2, 512], FP8, tag="t", name=f"t1b{blk}")
        t2 = tpool.tile([128, 8, 2, 512], FP8, tag="t", name=f"t2b{blk}")
        t3 = tpool.tile([128, 8, 2, 512], FP8, tag="t", name=f"t3b{blk}")
        t4 = tpool.tile([128, 8, 2, 512], FP8, tag="t", name=f"t4b{blk}")
        b11 = act_in[:, 0:8, :, nl]
        b21 = act_in[:, 8:16, :, nl]
        b12 = act_in[:, 0:8, :, nr]
        b22 = act_in[:, 8:16, :, nr]
        g = nc.gpsimd
        g.tensor_tensor(out=t1[:], in0=b12, in1=b11, op=ALU.subtract)
        g.tensor_tensor(out=t2[:], in0=b22, in1=t1[:], op=ALU.subtract)
        g.tensor_tensor(out=t3[:], in0=b22, in1=b12, op=ALU.subtract)
        g.tensor_tensor(out=t4[:], in0=t2[:], in1=b21, op=ALU.subtract)
        # rhs for products [P1, P2n, P3, P5n, P6, P7, P4]
        rhs = [lambda c: act_in[:, c, :, nl],
               lambda c: act_in[:, 8 + c, :, nl],
               lambda c: act_in[:, 8 + c, :, nr],
               lambda c: t1[:, c, :, :],
               lambda c: t2[:, c, :, :],
               lambda c: t3[:, c, :, :],
               lambda c: t4[:, c, :, :]]
        for r in range(16):
            t_top = thr_sb[:, r:r + 1]
            t_bot = thr_sb[:, 16 + r:17 + r]
            o_tl = act_out[:, r // 2, r % 2, nl]       # C11
            o_tr = act_out[:, r // 2, r % 2, nr]       # C12
            o_bl = act_out[:, 8 + r // 2, r % 2, nl]   # C21
            o_br = act_out[:, 8 + r // 2, r % 2, nr]   # C22
            pb = [psum_pool.tile([128, 512], FP32, tag="psum",
                                 name=f"sp{blk}_{r}_{p}") for p in range(7)]
            s0 = spool.tile([128, 512], FP32, tag="v", name=f"s0_{blk}{r}")
            v2 = spool.tile([128, 512], FP32, tag="v", name=f"v2_{blk}{r}")
            v3 = spool.tile([128, 512], FP32, tag="v", name=f"v3_{blk}{r}")
            v4 = spool.tile([128, 512], FP32, tag="v", name=f"v4_{blk}{r}")
            for ch in range(2):
                wt = wpool.tile([128, 7, 4, 2, 128], FP8, tag="ws")
                nc.sync.dma_start(out=wt[:], in_=wS[r, ch])
                for p in range(7):
                    for cc in range(4):
                        c = 4 * ch + cc
                        nc.tensor.matmul(
                            pb[p][:], lhsT=wt[:, p, cc, :, :], rhs=rhs[p](c),
                            start=(c == 0), stop=(c == 7), perf_mode=DR)
            # DVE reads at most one PSUM operand (the STT in1 port), so
            # P1 is copied to SBUF once and every combine chains off SBUF.
            nc.vector.tensor_scalar(               # s0 = P1
                out=s0[:], in0=pb[0][:], scalar1=0.0, scalar2=None,
                op0=ALU.add)
            nc.vector.scalar_tensor_tensor(        # C11 = [(P1 - t) >= -P2]
                out=o_tl, in0=s0[:], scalar=t_top, in1=pb[1][:],
                op0=ALU.subtract, op1=ALU.is_ge)
            nc.vector.scalar_tensor_tensor(        # V2 = U2 = P1 + P6
                out=v2[:], in0=s0[:], scalar=0.0, in1=pb[4][:],
                op0=ALU.bypass, op1=ALU.add)
            nc.vector.scalar_tensor_tensor(        # V3 = U2 + P7 - t_bot
                out=v3[:], in0=v2[:], scalar=t_bot, in1=pb[5][:],
                op0=ALU.subtract, op1=ALU.add)
            nc.vector.scalar_tensor_tensor(        # V4 = U2 + P3 - t_top
                out=v4[:], in0=v2[:], scalar=t_top, in1=pb[2][:],
                op0=ALU.subtract, op1=ALU.add)
            nc.vector.scalar_tensor_tensor(        # C12 = [V4 >= -P5]
                out=o_tr, in0=v4[:], scalar=0.0, in1=pb[3][:],
                op0=ALU.bypass, op1=ALU.is_ge)
            nc.vector.scalar_tensor_tensor(        # C21 = [V3 >= P4]
                out=o_bl, in0=v3[:], scalar=0.0, in1=pb[6][:],
                op0=ALU.bypass, op1=ALU.is_ge)
            nc.vector.scalar_tensor_tensor(        # C22 = [V3 >= -P5]
                out=o_br, in0=v3[:], scalar=0.0, in1=pb[3][:],
                op0=ALU.bypass, op1=ALU.is_ge)


def build_program(bc=BC):
    """Build the per-core Bass/Tile program (SPMD; identical on all cores)."""
    NT = bc // 512
    BT = bc // 128
    DR = mybir.MatmulPerfMode.DoubleRow

    nc = bacc.Bacc(None, target_bir_lowering=False, debug=False)

    a1h = nc.dram_tensor("a1h", [128, 3, 2, bc], FP8, kind="ExternalInput")
    w1 = nc.dram_tensor("w1dr", [32, 128, 3, 2, 128], FP8, kind="ExternalInput")
    wS2 = nc.dram_tensor("wS2", [16, 2, 128, 7, 4, 2, 128], FP8,
                         kind="ExternalInput")
    wS3 = nc.dram_tensor("wS3", [16, 2, 128, 7, 4, 2, 128], FP8,
                         kind="ExternalInput")
    w4 = nc.dram_tensor("w4dr", [128, 16, 2, 16], FP8, kind="ExternalInput")
    thrs = nc.dram_tensor("thrs", [128, 3, 32], FP32, kind="ExternalInput")
    c4 = nc.dram_tensor("c4", [16, 2], FP32, kind="ExternalInput")
    out = nc.dram_tensor("out", [128, bc // 128, OUT], FP32,
                         kind="ExternalOutput")

    with tile.TileContext(nc) as tc, ExitStack() as ctx:
        consts = ctx.enter_context(tc.tile_pool(name="consts", bufs=1))
        xpool = ctx.enter_context(tc.tile_pool(name="xpool", bufs=2))
        apool = ctx.enter_context(tc.tile_pool(name="apool", bufs=2))
        wpool = ctx.enter_context(tc.tile_pool(name="wpool", bufs=2))
        wspool = ctx.enter_context(tc.tile_pool(name="wspool", bufs=3))
        tpool = ctx.enter_context(tc.tile_pool(name="tpool", bufs=4))
        spool = ctx.enter_context(tc.tile_pool(name="spool", bufs=4))
        smpool = ctx.enter_context(tc.tile_pool(name="smpool", bufs=3))
        psum_pool = ctx.enter_context(
            tc.tile_pool(name="psum", bufs=8, space="PSUM"))

        thrs_sb = consts.tile([128, 3, 32], FP32, tag="thrs")
        c4_sb = consts.tile([16, 2], FP32, tag="c4")
        w4_sb = consts.tile([128, 16, 2, 16], FP8, tag="w4")
        ident = consts.tile([128, 128], FP32, tag="ident")
        h4 = consts.tile([16, bc], FP32, tag="h4")
        out_sb = consts.tile([128, BT, OUT], FP32, tag="outsb")
        thr1_sb = thrs_sb[:, 0, :]
        thr2_sb = thrs_sb[:, 1, :]
        thr3_sb = thrs_sb[:, 2, :]

        # ---- act1 = [x >= 0] {0,1} fp8 packed host-side, DMA'd direct.
        # act1 lives in the big act ring (slot0); act3 reuses the slot later
        # (its first write is ordered after act1's last read by Tile deps).
        act1big = apool.tile([128, 16, 2, bc], FP8, tag="actbig",
                             name="act1")
        act1 = act1big[:, 0:3, :, :]
        nc.sync.dma_start(out=act1[:, 0, :, :], in_=a1h[:, 0, :, :])
        nc.scalar.dma_start(out=act1[:, 1:3, :, :], in_=a1h[:, 1:3, :, :])

        # consts follow x on the rings; their data is needed much later
        nc.scalar.dma_start(out=thrs_sb[:], in_=thrs[:])
        nc.scalar.dma_start(out=c4_sb[:], in_=c4[:])
        nc.scalar.dma_start(out=w4_sb[:], in_=w4[:])
        make_identity(nc, ident[:])

        # PE warm-up: spans the act1/w1 DMA wait so layer 1 opens at speed.
        warm = consts.tile([128, 2, 128], FP8, tag="warm")
        nc.gpsimd.memset(warm[:], 0.0)
        wps = psum_pool.tile([128, 512], FP32, tag="psum", name="warmps")
        for _ in range(24):
            nc.tensor.matmul(
                wps[:, 0:128], lhsT=warm[:, :, 0:128], rhs=warm[:],
                start=True, stop=True, perf_mode=DR)

        # ---- layer 1 (768 -> 4096), classic DR, in TWO n-phases so that
        # the batch columns Strassen block 0 needs ({0:512, 1024:1536}) are
        # complete halfway through -> GpSimd builds L2's T-combos during
        # phase B and layer 2 opens with zero stall.
        act2 = apool.tile([128, 16, 2, bc], FP8, tag="actbig")
        for phase in range(2):
            for mt in range(32):
                wt = wpool.tile([128, 3, 2, 128], FP8, tag="w")
                nc.scalar.dma_start(out=wt[:], in_=w1[mt])
                pss = [psum_pool.tile([128, 512], FP32, tag="psum",
                                      name=f"l1ps{phase}_{mt}_{n}")
                       for n in range(2)]
                for i, n in enumerate((phase, phase + 2)):
                    for c in range(3):
                        nc.tensor.matmul(
                            pss[i][:],
                            lhsT=wt[:, c, :, :],
                            rhs=act1[:, c, :, 512 * n:512 * (n + 1)],
                            start=(c == 0), stop=(c == 2), perf_mode=DR)
                for i, n in enumerate((phase, phase + 2)):
                    nc.vector.tensor_scalar(
                        out=act2[:, mt // 2, mt % 2, 512 * n:512 * (n + 1)],
                        in0=pss[i][:], scalar1=thr1_sb[:, mt:mt + 1],
                        scalar2=None, op0=ALU.is_ge)

        # ---- layers 2+3: Strassen-Winograd ----
        act3 = apool.tile([128, 16, 2, bc], FP8, tag="actbig")
        _strassen_layer(nc, wspool, tpool, spool, psum_pool, act2, wS2,
                        thr2_sb, act3, bc)
        act4 = apool.tile([128, 16, 2, bc], FP8, tag="actbig")
        _strassen_layer(nc, wspool, tpool, spool, psum_pool, act3, wS3,
                        thr3_sb, act4, bc)

        # ---- layer 4: logits (M padded 10->16), affine folds BN+rowsum.
        sh = smpool.tile([128, BT, OUT], FP32, tag="sh", bufs=1)
        se = smpool.tile([128, BT], FP32, tag="se", bufs=1)
        ls = smpool.tile([128, BT], FP32, tag="ls", bufs=1)

        def _l4_softmax_head(g):
            # transpose group g's batch tiles + max/shift on DVE; runs one
            # n-group behind the L4 matmuls so the PE never stalls on it
            for bt in range(4 * g, 4 * g + 4):
                tp = psum_pool.tile([128, OUT], FP32, tag="psum",
                                    name=f"tp{bt}")
                nc.tensor.transpose(
                    tp[:], h4[0:OUT, 128 * bt:128 * (bt + 1)],
                    ident[0:OUT, 0:OUT])
                mx = smpool.tile([128, 1], FP32, tag="mx", name=f"mx{bt}")
                nc.vector.reduce_max(mx[:], tp[:], axis=mybir.AxisListType.X)
                nc.vector.tensor_scalar(
                    out=sh[:, bt, :], in0=tp[:], scalar1=mx[:], scalar2=None,
                    op0=ALU.subtract)
        for n in range(NT):
            ps4 = psum_pool.tile([16, 512], FP32, tag="psum", name=f"ps4_{n}")
            for c in range(16):
                nc.tensor.matmul(
                    ps4[:],
                    lhsT=w4_sb[:, c, :, :],
                    rhs=act4[:, c, :, 512 * n:512 * (n + 1)],
                    start=(c == 0),
                    stop=(c == 15),
                    perf_mode=DR,
                )
            # affine on DVE: out = in*scale + bias
            nc.vector.tensor_scalar(
                out=h4[:, 512 * n:512 * (n + 1)], in0=ps4[:],
                scalar1=c4_sb[:, 0:1], scalar2=c4_sb[:, 1:2],
                op0=ALU.mult, op1=ALU.add,
            )
        for g in range(NT):
            _l4_softmax_head(g)
        ex = smpool.tile([128, BT, OUT], FP32, tag="ex", bufs=1)
        for bt in range(BT):  # all Exp together: one ACT table load
            nc.scalar.activation(
                ex[:, bt, :], sh[:, bt, :], mybir.ActivationFunctionType.Exp,
                accum_out=se[:, bt:bt + 1])
        nc.scalar.activation(  # single Ln over all batch tiles
            ls[:], se[:], mybir.ActivationFunctionType.Ln)
        for bt in range(BT):
            nc.vector.tensor_scalar(
                out=out_sb[:, bt, :], in0=sh[:, bt, :],
                scalar1=ls[:, bt:bt + 1], scalar2=None,
                op0=ALU.subtract)

        # out dram is [128, BT, OUT] (partition-major, fully contiguous DMA);
        # the host reassembles batch order with a free transpose.
        nc.sync.dma_start(out=out[:], in_=out_sb[:])

    nc.compile()
    return nc


# --------------------------------------------------------------------------
# Host-side preparation
# --------------------------------------------------------------------------

def _pack_w_dr(ws_t):
    """[Fin, Fout] {-1,+1} -> [Mt, 128, C, 2, 128] fp8 DoubleRow layout.

    wdr[mt, ki, c, ko, mi] = ws_t[256*c + 128*ko + ki, 128*mt + mi]
    """
    fin, fout = ws_t.shape
    C, Mt = fin // 256, fout // 128
    w = ws_t.reshape(C, 2, 128, Mt, 128).transpose(3, 2, 0, 1, 4)
    return np.ascontiguousarray(w).astype(NP_FP8)


def _pack_strassen(ws):
    """[4096 out, 4096 in] +-1 -> [16, 128, 7, 8, 2, 128] fp8: the 7
    Winograd left-operand combos of W' = 2*ws in DoubleRow layout,
    product order [P1, P2n, P3, P5n, P6, P7, P4] (P2, P5 negated so the
    device combines become single is_ge scalar_tensor_tensor ops)."""
    Wp = 2.0 * ws
    W11, W12 = Wp[:2048, :2048], Wp[:2048, 2048:]
    W21, W22 = Wp[2048:, :2048], Wp[2048:, 2048:]
    S1 = W21 + W22
    S2 = S1 - W11
    S3 = W11 - W21
    S4 = W12 - S2
    Ls = [W11, -W12, S4, -S1, S2, S3, W22]
    # L[128r+mi, 256c+128i+ki] -> arr[r, p, c, ki, i, mi] -> [r, ki, p, c, i, mi]
    arr = np.stack(
        [L.reshape(16, 128, 8, 2, 128).transpose(0, 2, 4, 3, 1) for L in Ls],
        axis=1)  # [r, p, c, ki, i, mi]
    arr = arr.reshape(16, 7, 2, 4, 128, 2, 128).transpose(0, 2, 4, 1, 3, 5, 6)
    return np.ascontiguousarray(arr).astype(NP_FP8)  # [r, ch, ki, p, cc, i, mi]


def prepare_consts(inputs):
    """Fold sign(w), BN, bias and the 0/1-activation rowsum correction.

    The device computes, per layer, a_dev = [mmA~ >= thr] where
    mmA~ = W~sign @ a_dev_prev over {0,1} activations. Negative BN scales
    (alpha <= 0) are handled exactly by tracking a per-neuron flip bit
    (a_true = 1 - a_dev) that folds into the *next* layer's weight signs:
    with s~ = s * (1-2*flip_in), mm_full = 2*(s~ @ a_dev) - rowsum(s~)
    holds for any flip pattern. Thresholds use integer snapping (mmA is
    always an integer), making the device comparison tie-free/exact.
    Layers 2+3 carry weights W' = 2*W (Strassen packing), so their
    thresholds are doubled (still integers -> still tie-free).
    """
    consts = {}
    flip_in = np.zeros(IND)  # input layer: a_dev = [x >= 0] = ste_sign, exact
    for i in (1, 2, 3, 4):
        w = np.asarray(inputs[f"w{i}"]).astype(np.float64)
        b = np.asarray(inputs[f"b{i}"]).astype(np.float64)
        g = np.asarray(inputs[f"g{i}"]).astype(np.float64)
        be = np.asarray(inputs[f"be{i}"]).astype(np.float64)
        m = np.asarray(inputs[f"m{i}"]).astype(np.float64)
        v = np.asarray(inputs[f"v{i}"]).astype(np.float64)
        ws = np.where(w >= 0, 1.0, -1.0) * (1.0 - 2.0 * flip_in)  # [fo, fi]
        rowsum = ws.sum(axis=1)                                   # [fo]
        alpha = g / np.sqrt(v + EPS)
        if i < 4:
            # BN(mm_full + b) >= 0 with mm_full = 2*mmA - rowsum:
            #   alpha > 0:  a_true = [mmA >= u],  u = (m-b-be/a+rowsum)/2
            #   alpha < 0:  a_true = [mmA <= u] = 1 - [mmA >= floor(u)+1]
            #   alpha == 0: BN = be, constant sign
            u = (m - b - be / alpha_safe(alpha) + rowsum) / 2.0
            pos = alpha > 0
            thr = np.where(pos, np.ceil(u), np.floor(u) + 1.0)
            zero = alpha == 0
            if zero.any():
                # constant: a_true = [be >= 0]; force a_dev accordingly
                thr = np.where(zero & (be >= 0), -1e30, thr)
                thr = np.where(zero & (be < 0), 1e30, thr)
                pos = pos | zero
            flip_in = (~pos).astype(np.float64)
            scale = 1.0 if i == 1 else 2.0  # Strassen layers carry 2x weights
            consts.setdefault("_thrs", []).append(
                (scale * thr).reshape(32, 128).T.astype(np.float32))
            if i == 1:
                consts["w1dr"] = _pack_w_dr(ws.T)
            else:
                consts[f"wS{i}"] = _pack_strassen(ws)
        else:
            # logits = mmA*(2*alpha) + ((b - m - rowsum)*alpha + be), pad to 16
            scale = 2.0 * alpha
            beta = (b - m - rowsum) * alpha + be
            c4 = np.zeros((16, 2), np.float32)
            c4[:10, 0] = scale.astype(np.float32)
            c4[:10, 1] = beta.astype(np.float32)
            consts["c4"] = c4
            ws_t_pad = np.zeros((HID, 16), np.float64)
            ws_t_pad[:, :10] = ws.T
            # w4dr[ki, c, ko, m] = ws_t_pad[256*c + 128*ko + ki, m]
            w4 = ws_t_pad.reshape(16, 2, 128, 16).transpose(2, 0, 1, 3)
            consts["w4dr"] = np.ascontiguousarray(w4).astype(NP_FP8)
    consts["thrs"] = np.ascontiguousarray(
        np.stack(consts.pop("_thrs"), axis=1))  # [128, 3, 32]
    return consts


def alpha_safe(a):
    return np.where(a == 0, 1.0, a)


_PROG_CACHE = {}


def _get_program(bc=BC):
    if bc not in _PROG_CACHE:
        _PROG_CACHE[bc] = build_program(bc)
    return _PROG_CACHE[bc]


def kernel(**inputs):
    global LAST_RESULTS
    x = np.asarray(inputs["x"], np.float32)
    assert x.shape == (B, 784)
    consts = prepare_consts(inputs)
    # act1 = [x >= 0] in {0,1} fp8 packed host-side straight into the
    # feature-major device layout a1h[p, c, i, n] = a[256c+128i+p, n].
    a1_full = (x[:, :IND].T >= 0).astype(NP_FP8)  # [768, B]
    a1_full = a1_full.reshape(3, 2, 128, B).transpose(2, 0, 1, 3)

    nc = _get_program(BC)
    in_maps = []
    for c in range(N_CORES):
        m = {"a1h": np.ascontiguousarray(a1_full[:, :, :, c * BC:(c + 1) * BC])}
        m.update(consts)
        in_maps.append(m)

    res = run_bass_kernel_spmd(
        nc, in_maps, core_ids=list(range(N_CORES)), trace=TRACE,
        **TRACE_KWARGS)
    LAST_RESULTS = res
    # device out is [128, BT, 10] partition-major; restore batch order
    outs = [np.ascontiguousarray(r["out"].transpose(1, 0, 2).reshape(BC, OUT))
            for r in res.results]
    return np.concatenate(outs, axis=0)


# revision 15
# speedup vs baseline: 1.2016x; 1.2016x over previous
"""Binarized-MLP (BinaryNet) forward on 8 Trainium2 NeuronCores.

Reference computation (per nn_FC_large):
    h = sign(x[:, :768]) @ sign(w1).T + b1 ; BN1 ; -> sign
    h = sign(h) @ sign(w2).T + b2         ; BN2 ; -> sign
    h = sign(h) @ sign(w3).T + b3         ; BN3 ; -> sign
    h = sign(h) @ sign(w4).T + b4         ; BN4 ; log_softmax

Strategy (data parallel, batch 16384 -> 2048 rows/core):
  * Activations are a in {0,1} fp8 (a = [pre-act >= 0]); the identity
    sign_mm = 2*(Wsign @ a) - rowsum(Wsign) folds the rowsum into
    per-neuron integer thresholds, so every epilogue is one DVE op.
  * Matmuls run in fp8e4 with perf_mode=DoubleRow (K=256/instruction).
  * Layers 2+3 (4096x4096) use one level of Strassen-Winograd: 7 products
    of half-size blocks instead of 8 (PE work x7/8). Weight-side combos
    (host, values in {-8..8}) and activation-side combos T1..T4 (GpSimd,
    values in {-2..2}) are exact in fp8; products are integers well inside
    fp32, so the whole pipeline stays bit-exact w.r.t. the reference
    (weights carry a 2x so thresholds are simply doubled).
    The 7 output-side adds fold thresholds + compare into DVE
    scalar_tensor_tensor ops: e.g. C11 = [(P1 - t) >= (-P2)] with the
    (-P2) product negated host-side.
  * Layer-4 logits are PE-transposed to batch-major; log_softmax runs
    on-device (DVE/ACT) with Exp batched to avoid ACT table thrash.

Everything is hardcoded for x:[16384,784], layers 768->4096->4096->4096->10.
"""

import numpy as np
import ml_dtypes
from contextlib import ExitStack

import concourse.mybir as mybir
import concourse.tile as tile
from concourse import bacc
from concourse.bass_utils import run_bass_kernel_spmd
from concourse.masks import make_identity

FP32 = mybir.dt.float32
BF16 = mybir.dt.bfloat16
FP8 = mybir.dt.float8e4
NP_FP8 = ml_dtypes.float8_e4m3
NP_BF16 = ml_dtypes.bfloat16

EPS = 1e-5
B, IND, HID, OUT = 16384, 768, 4096, 10
N_CORES = 8
BC = B // N_CORES  # 2048 batch rows per core

# Knobs (test.py may flip TRACE before calling kernel()).
TRACE = False
TRACE_KWARGS = {}
LAST_RESULTS = None  # BassKernelResults of the most recent run

ALU = mybir.AluOpType


# --------------------------------------------------------------------------
# Device program
# --------------------------------------------------------------------------

def _emit_t_combos(nc, tpool, act_in, blk, lname):
    """Winograd activation-side combos for one 512-col block, on DVE
    (values in {-2..2}, exact in fp8). Emitted a block AHEAD of use so the
    PE never waits on them."""
    nl = slice(512 * blk, 512 * blk + 512)
    nr = slice(1024 + 512 * blk, 1536 + 512 * blk)
    t1 = tpool.tile([128, 8, 2, 512], FP8, tag="t", name=f"t1{lname}{blk}")
    t2 = tpool.tile([128, 8, 2, 512], FP8, tag="t", name=f"t2{lname}{blk}")
    t3 = tpool.tile([128, 8, 2, 512], FP8, tag="t", name=f"t3{lname}{blk}")
    t4 = tpool.tile([128, 8, 2, 512], FP8, tag="t", name=f"t4{lname}{blk}")
    b11 = act_in[:, 0:8, :, nl]
    b21 = act_in[:, 8:16, :, nl]
    b12 = act_in[:, 0:8, :, nr]
    b22 = act_in[:, 8:16, :, nr]
    v = nc.vector
    v.tensor_tensor(out=t1[:], in0=b12, in1=b11, op=ALU.subtract)
    v.tensor_tensor(out=t2[:], in0=b22, in1=t1[:], op=ALU.subtract)
    v.tensor_tensor(out=t3[:], in0=b22, in1=b12, op=ALU.subtract)
    v.tensor_tensor(out=t4[:], in0=t2[:], in1=b21, op=ALU.subtract)
    return (t1, t2, t3, t4)


def _strassen_layer(nc, wpool, tpool, spool, psum_pool, act_in, wS, thr_sb,
                    act_out, bc, t_tiles, next_t=None):
    """One 4096x4096 binarized layer via 1-level Strassen-Winograd.

    act_in : SBUF AP [128, 16, 2, bc] fp8 {0,1}
    wS     : DRAM [16, 2, 128, 7, 4, 2, 128] fp8 (7 left-combos, DR layout)
    thr_sb : SBUF [128, 32] fp32 (2x integer-snapped thresholds)
    act_out: SBUF AP [128, 16, 2, bc] fp8
    t_tiles: [t-tuple for blk0, t-tuple for blk1] (emitted ahead)
    next_t : callback(blk) -> emits the NEXT consumer's T-combos once this
             layer's block blk outputs are complete.
    """
    DR = mybir.MatmulPerfMode.DoubleRow
    for blk in range(2):
        nl = slice(512 * blk, 512 * blk + 512)
        nr = slice(1024 + 512 * blk, 1536 + 512 * blk)
        t1, t2, t3, t4 = t_tiles[blk]
        # rhs for products [P1, P2n, P3, P5n, P6, P7, P4]
        rhs = [lambda c: act_in[:, c, :, nl],
               lambda c: act_in[:, 8 + c, :, nl],
               lambda c: act_in[:, 8 + c, :, nr],
               lambda c: t1[:, c, :, :],
               lambda c: t2[:, c, :, :],
               lambda c: t3[:, c, :, :],
               lambda c: t4[:, c, :, :]]
        for r in range(16):
            t_top = thr_sb[:, r:r + 1]
            t_bot = thr_sb[:, 16 + r:17 + r]
            o_tl = act_out[:, r // 2, r % 2, nl]       # C11
            o_tr = act_out[:, r // 2, r % 2, nr]       # C12
            o_bl = act_out[:, 8 + r // 2, r % 2, nl]   # C21
            o_br = act_out[:, 8 + r // 2, r % 2, nr]   # C22
            pb = [psum_pool.tile([128, 512], FP32, tag="psum",
                                 name=f"sp{blk}_{r}_{p}") for p in range(7)]
            s0 = spool.tile([128, 512], FP32, tag="v", name=f"s0_{blk}{r}")
            v2 = spool.tile([128, 512], FP32, tag="v", name=f"v2_{blk}{r}")
            v3 = spool.tile([128, 512], FP32, tag="v", name=f"v3_{blk}{r}")
            v4 = spool.tile([128, 512], FP32, tag="v", name=f"v4_{blk}{r}")
            for ch in range(2):
                wt = wpool.tile([128, 7, 4, 2, 128], FP8, tag="ws")
                nc.sync.dma_start(out=wt[:], in_=wS[r, ch])
                for p in range(7):
                    for cc in range(4):
                        c = 4 * ch + cc
                        nc.tensor.matmul(
                            pb[p][:], lhsT=wt[:, p, cc, :, :], rhs=rhs[p](c),
                            start=(c == 0), stop=(c == 7), perf_mode=DR)
            # DVE reads at most one PSUM operand (the STT in1 port), so
            # P1 is copied to SBUF once and every combine chains off SBUF.
            nc.vector.tensor_scalar(               # s0 = P1
                out=s0[:], in0=pb[0][:], scalar1=0.0, scalar2=None,
                op0=ALU.add)
            nc.vector.scalar_tensor_tensor(        # C11 = [(P1 - t) >= -P2]
                out=o_tl, in0=s0[:], scalar=t_top, in1=pb[1][:],
                op0=ALU.subtract, op1=ALU.is_ge)
            nc.vector.scalar_tensor_tensor(        # V2 = U2 = P1 + P6
                out=v2[:], in0=s0[:], scalar=0.0, in1=pb[4][:],
                op0=ALU.bypass, op1=ALU.add)
            nc.vector.scalar_tensor_tensor(        # V3 = U2 + P7 - t_bot
                out=v3[:], in0=v2[:], scalar=t_bot, in1=pb[5][:],
                op0=ALU.subtract, op1=ALU.add)
            nc.vector.scalar_tensor_tensor(        # V4 = U2 + P3 - t_top
                out=v4[:], in0=v2[:], scalar=t_top, in1=pb[2][:],
                op0=ALU.subtract, op1=ALU.add)
            nc.vector.scalar_tensor_tensor(        # C12 = [V4 >= -P5]
                out=o_tr, in0=v4[:], scalar=0.0, in1=pb[3][:],
                op0=ALU.bypass, op1=ALU.is_ge)
            nc.vector.scalar_tensor_tensor(        # C21 = [V3 >= P4]
                out=o_bl, in0=v3[:], scalar=0.0, in1=pb[6][:],
                op0=ALU.bypass, op1=ALU.is_ge)
            nc.vector.scalar_tensor_tensor(        # C22 = [V3 >= -P5]
                out=o_br, in0=v3[:], scalar=0.0, in1=pb[3][:],
                op0=ALU.bypass, op1=ALU.is_ge)
        if next_t is not None:
            next_t(blk)


def build_program(bc=BC):
    """Build the per-core Bass/Tile program (SPMD; identical on all cores)."""
    NT = bc // 512
    BT = bc // 128
    DR = mybir.MatmulPerfMode.DoubleRow

    nc = bacc.Bacc(None, target_bir_lowering=False, debug=False)

    a1h = nc.dram_tensor("a1h", [128, 3, 2, bc], FP8, kind="ExternalInput")
    w1 = nc.dram_tensor("w1dr", [32, 128, 3, 2, 128], FP8, kind="ExternalInput")
    wS2 = nc.dram_tensor("wS2", [16, 2, 128, 7, 4, 2, 128], FP8,
                         kind="ExternalInput")
    wS3 = nc.dram_tensor("wS3", [16, 2, 128, 7, 4, 2, 128], FP8,
                         kind="ExternalInput")
    w4 = nc.dram_tensor("w4dr", [128, 16, 2, 16], FP8, kind="ExternalInput")
    thrs = nc.dram_tensor("thrs", [128, 3, 32], FP32, kind="ExternalInput")
    c4 = nc.dram_tensor("c4", [16, 2], FP32, kind="ExternalInput")
    out = nc.dram_tensor("out", [128, bc // 128, OUT], FP32,
                         kind="ExternalOutput")

    with tile.TileContext(nc) as tc, ExitStack() as ctx:
        consts = ctx.enter_context(tc.tile_pool(name="consts", bufs=1))
        xpool = ctx.enter_context(tc.tile_pool(name="xpool", bufs=2))
        apool = ctx.enter_context(tc.tile_pool(name="apool", bufs=2))
        wpool = ctx.enter_context(tc.tile_pool(name="wpool", bufs=2))
        wspool = ctx.enter_context(tc.tile_pool(name="wspool", bufs=2))
        tpool = ctx.enter_context(tc.tile_pool(name="tpool", bufs=5))
        spool = ctx.enter_context(tc.tile_pool(name="spool", bufs=4))
        smpool = ctx.enter_context(tc.tile_pool(name="smpool", bufs=3))
        psum_pool = ctx.enter_context(
            tc.tile_pool(name="psum", bufs=8, space="PSUM"))

        thrs_sb = consts.tile([128, 3, 32], FP32, tag="thrs")
        c4_sb = consts.tile([16, 2], FP32, tag="c4")
        w4_sb = consts.tile([128, 16, 2, 16], FP8, tag="w4")
        ident = consts.tile([128, 128], FP32, tag="ident")
        h4 = consts.tile([16, bc], FP32, tag="h4")
        out_sb = consts.tile([128, BT, OUT], FP32, tag="outsb")
        thr1_sb = thrs_sb[:, 0, :]
        thr2_sb = thrs_sb[:, 1, :]
        thr3_sb = thrs_sb[:, 2, :]

        # ---- act1 = [x >= 0] {0,1} fp8 packed host-side, DMA'd direct.
        # act1 lives in the big act ring (slot0); act3 reuses the slot later
        # (its first write is ordered after act1's last read by Tile deps).
        act1big = apool.tile([128, 16, 2, bc], FP8, tag="actbig",
                             name="act1")
        act1 = act1big[:, 0:3, :, :]
        nc.sync.dma_start(out=act1[:, 0, :, :], in_=a1h[:, 0, :, :])
        nc.scalar.dma_start(out=act1[:, 1:3, :, :], in_=a1h[:, 1:3, :, :])

        # consts follow x on the rings; their data is needed much later
        nc.scalar.dma_start(out=thrs_sb[:], in_=thrs[:])
        nc.scalar.dma_start(out=c4_sb[:], in_=c4[:])
        nc.scalar.dma_start(out=w4_sb[:], in_=w4[:])
        make_identity(nc, ident[:])

        # PE warm-up: spans the act1/w1 DMA wait so layer 1 opens at speed.
        warm = consts.tile([128, 2, 128], FP8, tag="warm")
        nc.gpsimd.memset(warm[:], 0.0)
        wps = psum_pool.tile([128, 512], FP32, tag="psum", name="warmps")
        for _ in range(24):
            nc.tensor.matmul(
                wps[:, 0:128], lhsT=warm[:, :, 0:128], rhs=warm[:],
                start=True, stop=True, perf_mode=DR)

        # ---- layer 1 (768 -> 4096), classic DR, in TWO n-phases so that
        # the batch columns Strassen block 0 needs ({0:512, 1024:1536}) are
        # complete halfway through -> GpSimd builds L2's T-combos during
        # phase B and layer 2 opens with zero stall.
        act2 = apool.tile([128, 16, 2, bc], FP8, tag="actbig")
        t2_b0 = None
        for phase in range(2):
            for mt in range(32):
                wt = wpool.tile([128, 3, 2, 128], FP8, tag="w")
                nc.scalar.dma_start(out=wt[:], in_=w1[mt])
                pss = [psum_pool.tile([128, 512], FP32, tag="psum",
                                      name=f"l1ps{phase}_{mt}_{n}")
                       for n in range(2)]
                for i, n in enumerate((phase, phase + 2)):
                    for c in range(3):
                        nc.tensor.matmul(
                            pss[i][:],
                            lhsT=wt[:, c, :, :],
                            rhs=act1[:, c, :, 512 * n:512 * (n + 1)],
                            start=(c == 0), stop=(c == 2), perf_mode=DR)
                for i, n in enumerate((phase, phase + 2)):
                    nc.vector.tensor_scalar(
                        out=act2[:, mt // 2, mt % 2, 512 * n:512 * (n + 1)],
                        in0=pss[i][:], scalar1=thr1_sb[:, mt:mt + 1],
                        scalar2=None, op0=ALU.is_ge)
            if phase == 0:
                t2_b0 = _emit_t_combos(nc, tpool, act2, 0, "L2")

        # ---- layers 2+3: Strassen-Winograd ----
        # T-combos are emitted one block ahead: L2/blk0's during L1 phase B,
        # L2/blk1's at the end of L2/blk0, L3/blk0's at the end of L2/blk1...
        act3 = apool.tile([128, 16, 2, bc], FP8, tag="actbig")
        act4 = apool.tile([128, 16, 2, bc], FP8, tag="actbig")
        t2_tiles = [t2_b0, None]
        t3_tiles = [None, None]

        def next_t_l2(blk):
            if blk == 0:
                t2_tiles[1] = _emit_t_combos(nc, tpool, act2, 1, "L2")
            else:
                t3_tiles[0] = _emit_t_combos(nc, tpool, act3, 0, "L3")

        def next_t_l3(blk):
            if blk == 0:
                t3_tiles[1] = _emit_t_combos(nc, tpool, act3, 1, "L3")

        _strassen_layer(nc, wspool, tpool, spool, psum_pool, act2, wS2,
                        thr2_sb, act3, bc, t2_tiles, next_t=next_t_l2)
        _strassen_layer(nc, wspool, tpool, spool, psum_pool, act3, wS3,
                        thr3_sb, act4, bc, t3_tiles, next_t=next_t_l3)

        # ---- layer 4: logits (M padded 10->16), affine folds BN+rowsum.
        sh = smpool.tile([128, BT, OUT], FP32, tag="sh", bufs=1)
        se = smpool.tile([128, BT], FP32, tag="se", bufs=1)
        ls = smpool.tile([128, BT], FP32, tag="ls", bufs=1)

        def _l4_softmax_head(g):
            # transpose group g's batch tiles + max/shift on DVE; runs one
            # n-group behind the L4 matmuls so the PE never stalls on it
            for bt in range(4 * g, 4 * g + 4):
                tp = psum_pool.tile([128, OUT], FP32, tag="psum",
                                    name=f"tp{bt}")
                nc.tensor.transpose(
                    tp[:], h4[0:OUT, 128 * bt:128 * (bt + 1)],
                    ident[0:OUT, 0:OUT])
                mx = smpool.tile([128, 1], FP32, tag="mx", name=f"mx{bt}")
                nc.vector.reduce_max(mx[:], tp[:], axis=mybir.AxisListType.X)
                nc.vector.tensor_scalar(
                    out=sh[:, bt, :], in0=tp[:], scalar1=mx[:], scalar2=None,
                    op0=ALU.subtract)
        for n in range(NT):
            ps4 = psum_pool.tile([16, 512], FP32, tag="psum", name=f"ps4_{n}")
            for c in range(16):
                nc.tensor.matmul(
                    ps4[:],
                    lhsT=w4_sb[:, c, :, :],
                    rhs=act4[:, c, :, 512 * n:512 * (n + 1)],
                    start=(c == 0),
                    stop=(c == 15),
                    perf_mode=DR,
                )
            # affine on DVE: out = in*scale + bias
            nc.vector.tensor_scalar(
                out=h4[:, 512 * n:512 * (n + 1)], in0=ps4[:],
                scalar1=c4_sb[:, 0:1], scalar2=c4_sb[:, 1:2],
                op0=ALU.mult, op1=ALU.add,
            )
        for g in range(NT):
            _l4_softmax_head(g)
        ex = smpool.tile([128, BT, OUT], FP32, tag="ex", bufs=1)
        for bt in range(BT):  # all Exp together: one ACT table load
            nc.scalar.activation(
                ex[:, bt, :], sh[:, bt, :], mybir.ActivationFunctionType.Exp,
                accum_out=se[:, bt:bt + 1])
        nc.scalar.activation(  # single Ln over all batch tiles
            ls[:], se[:], mybir.ActivationFunctionType.Ln)
        for bt in range(BT):
            nc.vector.tensor_scalar(
                out=out_sb[:, bt, :], in0=sh[:, bt, :],
                scalar1=ls[:, bt:bt + 1], scalar2=None,
                op0=ALU.subtract)

        # out dram is [128, BT, OUT] (partition-major, fully contiguous DMA);
        # the host reassembles batch order with a free transpose.
        nc.sync.dma_start(out=out[:], in_=out_sb[:])

    nc.compile()
    return nc


# --------------------------------------------------------------------------
# Host-side preparation
# --------------------------------------------------------------------------

def _pack_w_dr(ws_t):
    """[Fin, Fout] {-1,+1} -> [Mt, 128, C, 2, 128] fp8 DoubleRow layout.

    wdr[mt, ki, c, ko, mi] = ws_t[256*c + 128*ko + ki, 128*mt + mi]
    """
    fin, fout = ws_t.shape
    C, Mt = fin // 256, fout // 128
    w = ws_t.reshape(C, 2, 128, Mt, 128).transpose(3, 2, 0, 1, 4)
    return np.ascontiguousarray(w).astype(NP_FP8)


def _pack_strassen(ws):
    """[4096 out, 4096 in] +-1 -> [16, 128, 7, 8, 2, 128] fp8: the 7
    Winograd left-operand combos of W' = 2*ws in DoubleRow layout,
    product order [P1, P2n, P3, P5n, P6, P7, P4] (P2, P5 negated so the
    device combines become single is_ge scalar_tensor_tensor ops)."""
    Wp = 2.0 * ws
    W11, W12 = Wp[:2048, :2048], Wp[:2048, 2048:]
    W21, W22 = Wp[2048:, :2048], Wp[2048:, 2048:]
    S1 = W21 + W22
    S2 = S1 - W11
    S3 = W11 - W21
    S4 = W12 - S2
    Ls = [W11, -W12, S4, -S1, S2, S3, W22]
    # L[128r+mi, 256c+128i+ki] -> arr[r, p, c, ki, i, mi] -> [r, ki, p, c, i, mi]
    arr = np.stack(
        [L.reshape(16, 128, 8, 2, 128).transpose(0, 2, 4, 3, 1) for L in Ls],
        axis=1)  # [r, p, c, ki, i, mi]
    arr = arr.reshape(16, 7, 2, 4, 128, 2, 128).transpose(0, 2, 4, 1, 3, 5, 6)
    return np.ascontiguousarray(arr).astype(NP_FP8)  # [r, ch, ki, p, cc, i, mi]


def prepare_consts(inputs):
    """Fold sign(w), BN, bias and the 0/1-activation rowsum correction.

    The device computes, per layer, a_dev = [mmA~ >= thr] where
    mmA~ = W~sign @ a_dev_prev over {0,1} activations. Negative BN scales
    (alpha <= 0) are handled exactly by tracking a per-neuron flip bit
    (a_true = 1 - a_dev) that folds into the *next* layer's weight signs:
    with s~ = s * (1-2*flip_in), mm_full = 2*(s~ @ a_dev) - rowsum(s~)
    holds for any flip pattern. Thresholds use integer snapping (mmA is
    always an integer), making the device comparison tie-free/exact.
    Layers 2+3 carry weights W' = 2*W (Strassen packing), so their
    thresholds are doubled (still integers -> still tie-free).
    """
    consts = {}
    flip_in = np.zeros(IND)  # input layer: a_dev = [x >= 0] = ste_sign, exact
    for i in (1, 2, 3, 4):
        w = np.asarray(inputs[f"w{i}"]).astype(np.float64)
        b = np.asarray(inputs[f"b{i}"]).astype(np.float64)
        g = np.asarray(inputs[f"g{i}"]).astype(np.float64)
        be = np.asarray(inputs[f"be{i}"]).astype(np.float64)
        m = np.asarray(inputs[f"m{i}"]).astype(np.float64)
        v = np.asarray(inputs[f"v{i}"]).astype(np.float64)
        ws = np.where(w >= 0, 1.0, -1.0) * (1.0 - 2.0 * flip_in)  # [fo, fi]
        rowsum = ws.sum(axis=1)                                   # [fo]
        alpha = g / np.sqrt(v + EPS)
        if i < 4:
            # BN(mm_full + b) >= 0 with mm_full = 2*mmA - rowsum:
            #   alpha > 0:  a_true = [mmA >= u],  u = (m-b-be/a+rowsum)/2
            #   alpha < 0:  a_true = [mmA <= u] = 1 - [mmA >= floor(u)+1]
            #   alpha == 0: BN = be, constant sign
            u = (m - b - be / alpha_safe(alpha) + rowsum) / 2.0
            pos = alpha > 0
            thr = np.where(pos, np.ceil(u), np.floor(u) + 1.0)
            zero = alpha == 0
            if zero.any():
                # constant: a_true = [be >= 0]; force a_dev accordingly
                thr = np.where(zero & (be >= 0), -1e30, thr)
                thr = np.where(zero & (be < 0), 1e30, thr)
                pos = pos | zero
            flip_in = (~pos).astype(np.float64)
            scale = 1.0 if i == 1 else 2.0  # Strassen layers carry 2x weights
            consts.setdefault("_thrs", []).append(
                (scale * thr).reshape(32, 128).T.astype(np.float32))
            if i == 1:
                consts["w1dr"] = _pack_w_dr(ws.T)
            else:
                consts[f"wS{i}"] = _pack_strassen(ws)
        else:
            # logits = mmA*(2*alpha) + ((b - m - rowsum)*alpha + be), pad to 16
            scale = 2.0 * alpha
            beta = (b - m - rowsum) * alpha + be
            c4 = np.zeros((16, 2), np.float32)
            c4[:10, 0] = scale.astype(np.float32)
            c4[:10, 1] = beta.astype(np.float32)
            consts["c4"] = c4
            ws_t_pad = np.zeros((HID, 16), np.float64)
            ws_t_pad[:, :10] = ws.T
            # w4dr[ki, c, ko, m] = ws_t_pad[256*c + 128*ko + ki, m]
            w4 = ws_t_pad.reshape(16, 2, 128, 16).transpose(2, 0, 1, 3)
            consts["w4dr"] = np.ascontiguousarray(w4).astype(NP_FP8)
    consts["thrs"] = np.ascontiguousarray(
        np.stack(consts.pop("_thrs"), axis=1))  # [128, 3, 32]
    return consts


def alpha_safe(a):
    return np.where(a == 0, 1.0, a)


_PROG_CACHE = {}


def _get_program(bc=BC):
    if bc not in _PROG_CACHE:
        _PROG_CACHE[bc] = build_program(bc)
    return _PROG_CACHE[bc]


def kernel(**inputs):
    global LAST_RESULTS
    x = np.asarray(inputs["x"], np.float32)
    assert x.shape == (B, 784)
    consts = prepare_consts(inputs)
    # act1 = [x >= 0] in {0,1} fp8 packed host-side straight into the
    # feature-major device layout a1h[p, c, i, n] = a[256c+128i+p, n].
    a1_full = (x[:, :IND].T >= 0).astype(NP_FP8)  # [768, B]
    a1_full = a1_full.reshape(3, 2, 128, B).transpose(2, 0, 1, 3)

    nc = _get_program(BC)
    in_maps = []
    for c in range(N_CORES):
        m = {"a1h": np.ascontiguousarray(a1_full[:, :, :, c * BC:(c + 1) * BC])}
        m.update(consts)
        in_maps.append(m)

    res = run_bass_kernel_spmd(
        nc, in_maps, core_ids=list(range(N_CORES)), trace=TRACE,
        **TRACE_KWARGS)
    LAST_RESULTS = res
    # device out is [128, BT, 10] partition-major; restore batch order
    outs = [np.ascontiguousarray(r["out"].transpose(1, 0, 2).reshape(BC, OUT))
            for r in res.results]
    return np.concatenate(outs, axis=0)


# revision 18
# speedup vs baseline: 1.2269x; 1.0210x over previous
"""Binarized-MLP (BinaryNet) forward on 8 Trainium2 NeuronCores.

Reference computation (per nn_FC_large):
    h = sign(x[:, :768]) @ sign(w1).T + b1 ; BN1 ; -> sign
    h = sign(h) @ sign(w2).T + b2         ; BN2 ; -> sign
    h = sign(h) @ sign(w3).T + b3         ; BN3 ; -> sign
    h = sign(h) @ sign(w4).T + b4         ; BN4 ; log_softmax

Strategy (data parallel, batch 16384 -> 2048 rows/core):
  * Activations are a in {0,1} fp8 (a = [pre-act >= 0]); the identity
    sign_mm = 2*(Wsign @ a) - rowsum(Wsign) folds the rowsum into
    per-neuron integer thresholds, so every epilogue is one DVE op.
  * Matmuls run in fp8e4 with perf_mode=DoubleRow (K=256/instruction).
  * Layers 2+3 (4096x4096) use one level of Strassen-Winograd: 7 products
    of half-size blocks instead of 8 (PE work x7/8). Weight-side combos
    (host, values in {-8..8}) and activation-side combos T1..T4 (GpSimd,
    values in {-2..2}) are exact in fp8; products are integers well inside
    fp32, so the whole pipeline stays bit-exact w.r.t. the reference
    (weights carry a 2x so thresholds are simply doubled).
    The 7 output-side adds fold thresholds + compare into DVE
    scalar_tensor_tensor ops: e.g. C11 = [(P1 - t) >= (-P2)] with the
    (-P2) product negated host-side.
  * Layer-4 logits are PE-transposed to batch-major; log_softmax runs
    on-device (DVE/ACT) with Exp batched to avoid ACT table thrash.

Everything is hardcoded for x:[16384,784], layers 768->4096->4096->4096->10.
"""

import numpy as np
import ml_dtypes
from contextlib import ExitStack

import concourse.mybir as mybir
import concourse.tile as tile
from concourse import bacc
from concourse.bass_utils import run_bass_kernel_spmd
from concourse.masks import make_identity

FP32 = mybir.dt.float32
BF16 = mybir.dt.bfloat16
FP8 = mybir.dt.float8e4
NP_FP8 = ml_dtypes.float8_e4m3
NP_BF16 = ml_dtypes.bfloat16

EPS = 1e-5
B, IND, HID, OUT = 16384, 768, 4096, 10
N_CORES = 8
BC = B // N_CORES  # 2048 batch rows per core

# Knobs (test.py may flip TRACE before calling kernel()).
TRACE = False
TRACE_KWARGS = {}
LAST_RESULTS = None  # BassKernelResults of the most recent run

ALU = mybir.AluOpType


# --------------------------------------------------------------------------
# Device program
# --------------------------------------------------------------------------

def _emit_t12(nc, tpool, act_in, blk, lname):
    """t1 = B12-B11, t2 = B22-t1 on GpSimd (slow but emitted a whole block
    ahead into fresh pool slots, so it never gates the PE)."""
    nl = slice(512 * blk, 512 * blk + 512)
    nr = slice(1024 + 512 * blk, 1536 + 512 * blk)
    t1 = tpool.tile([128, 8, 2, 512], FP8, tag="t", name=f"t1{lname}{blk}")
    t2 = tpool.tile([128, 8, 2, 512], FP8, tag="t", name=f"t2{lname}{blk}")
    g = nc.gpsimd
    g.tensor_tensor(out=t1[:], in0=act_in[:, 0:8, :, nr],
                    in1=act_in[:, 0:8, :, nl], op=ALU.subtract)
    g.tensor_tensor(out=t2[:], in0=act_in[:, 8:16, :, nr], in1=t1[:],
                    op=ALU.subtract)
    return [t1, t2]


def _emit_t3(nc, tpool, act_in, blk, lname):
    """t3 = B22-B12 on DVE (fast; emitted mid-previous-block)."""
    nl = slice(512 * blk, 512 * blk + 512)
    nr = slice(1024 + 512 * blk, 1536 + 512 * blk)
    t3 = tpool.tile([128, 8, 2, 512], FP8, tag="t", name=f"t3{lname}{blk}")
    nc.vector.tensor_tensor(out=t3[:], in0=act_in[:, 8:16, :, nr],
                            in1=act_in[:, 0:8, :, nr], op=ALU.subtract)
    return t3


def _emit_t4h(nc, tpool, act_in, blk, t2, half, lname):
    """One k-half of t4 = t2 - B21 on DVE (own small pool ring)."""
    nl = slice(512 * blk, 512 * blk + 512)
    cs = slice(4 * half, 4 * half + 4)
    t4 = tpool.tile([128, 4, 2, 512], FP8, tag="t4", bufs=3,
                    name=f"t4{lname}{blk}h{half}")
    nc.vector.tensor_tensor(out=t4[:], in0=t2[:, cs, :, :],
                            in1=act_in[:, 8 + 4 * half:12 + 4 * half, :, nl],
                            op=ALU.subtract)
    return t4


def _strassen_layer(nc, wpool, tpool, spool, psum_pool, act_in, wS, thr_sb,
                    act_out, bc, t_tiles, hooks=None):
    """One 4096x4096 binarized layer via 1-level Strassen-Winograd.

    act_in : SBUF AP [128, 16, 2, bc] fp8 {0,1}
    wS     : DRAM [16, 2, 128, 7, 4, 2, 128] fp8 (7 left-combos, DR layout)
    thr_sb : SBUF [128, 32] fp32 (2x integer-snapped thresholds)
    act_out: SBUF AP [128, 16, 2, bc] fp8
    t_tiles: [t-list for blk0, t-list for blk1] (emitted ahead)
    hooks  : dict (blk, r) -> callback() emitting a future block's T-combos
             at a point where its inputs are ready and DVE/GpSimd have slack.
    """
    DR = mybir.MatmulPerfMode.DoubleRow
    for blk in range(2):
        nl = slice(512 * blk, 512 * blk + 512)
        nr = slice(1024 + 512 * blk, 1536 + 512 * blk)
        t1, t2, t3, t4a, t4b = t_tiles[blk]  # filled ahead by hooks
        # rhs for products [P1, P2n, P3, P5n, P6, P7, P4]
        rhs = [lambda c: act_in[:, c, :, nl],
               lambda c: act_in[:, 8 + c, :, nl],
               lambda c: act_in[:, 8 + c, :, nr],
               lambda c: t1[:, c, :, :],
               lambda c: t2[:, c, :, :],
               lambda c: t3[:, c, :, :],
               lambda c: (t4a if c < 4 else t4b)[:, c % 4, :, :]]
        for r in range(16):
            t_top = thr_sb[:, r:r + 1]
            t_bot = thr_sb[:, 16 + r:17 + r]
            o_tl = act_out[:, r // 2, r % 2, nl]       # C11
            o_tr = act_out[:, r // 2, r % 2, nr]       # C12
            o_bl = act_out[:, 8 + r // 2, r % 2, nl]   # C21
            o_br = act_out[:, 8 + r // 2, r % 2, nr]   # C22
            pb = [psum_pool.tile([128, 512], FP32, tag="psum",
                                 name=f"sp{blk}_{r}_{p}") for p in range(7)]
            s0 = spool.tile([128, 512], FP32, tag="v", name=f"s0_{blk}{r}")
            v2 = spool.tile([128, 512], FP32, tag="v", name=f"v2_{blk}{r}")
            v3 = spool.tile([128, 512], FP32, tag="v", name=f"v3_{blk}{r}")
            v4 = spool.tile([128, 512], FP32, tag="v", name=f"v4_{blk}{r}")
            for q in range(4):
                wt = wpool.tile([128, 7, 2, 2, 128], FP8, tag="ws")
                nc.sync.dma_start(out=wt[:], in_=wS[r, q])
                for p in range(7):
                    for cc in range(2):
                        c = 2 * q + cc
                        nc.tensor.matmul(
                            pb[p][:], lhsT=wt[:, p, cc, :, :], rhs=rhs[p](c),
                            start=(c == 0), stop=(c == 7), perf_mode=DR)
            # DVE reads at most one PSUM operand (the STT in1 port), so
            # P1 is copied to SBUF once and every combine chains off SBUF.
            nc.vector.tensor_scalar(               # s0 = P1
                out=s0[:], in0=pb[0][:], scalar1=0.0, scalar2=None,
                op0=ALU.add)
            nc.vector.scalar_tensor_tensor(        # C11 = [(P1 - t) >= -P2]
                out=o_tl, in0=s0[:], scalar=t_top, in1=pb[1][:],
                op0=ALU.subtract, op1=ALU.is_ge)
            nc.vector.scalar_tensor_tensor(        # V2 = U2 = P1 + P6
                out=v2[:], in0=s0[:], scalar=0.0, in1=pb[4][:],
                op0=ALU.bypass, op1=ALU.add)
            nc.vector.scalar_tensor_tensor(        # V3 = U2 + P7 - t_bot
                out=v3[:], in0=v2[:], scalar=t_bot, in1=pb[5][:],
                op0=ALU.subtract, op1=ALU.add)
            nc.vector.scalar_tensor_tensor(        # V4 = U2 + P3 - t_top
                out=v4[:], in0=v2[:], scalar=t_top, in1=pb[2][:],
                op0=ALU.subtract, op1=ALU.add)
            nc.vector.scalar_tensor_tensor(        # C12 = [V4 >= -P5]
                out=o_tr, in0=v4[:], scalar=0.0, in1=pb[3][:],
                op0=ALU.bypass, op1=ALU.is_ge)
            nc.vector.scalar_tensor_tensor(        # C21 = [V3 >= P4]
                out=o_bl, in0=v3[:], scalar=0.0, in1=pb[6][:],
                op0=ALU.bypass, op1=ALU.is_ge)
            nc.vector.scalar_tensor_tensor(        # C22 = [V3 >= -P5]
                out=o_br, in0=v3[:], scalar=0.0, in1=pb[3][:],
                op0=ALU.bypass, op1=ALU.is_ge)
            if hooks and (blk, r) in hooks:
                hooks[(blk, r)]()


def build_program(bc=BC):
    """Build the per-core Bass/Tile program (SPMD; identical on all cores)."""
    NT = bc // 512
    BT = bc // 128
    DR = mybir.MatmulPerfMode.DoubleRow

    nc = bacc.Bacc(None, target_bir_lowering=False, debug=False)

    a1h = nc.dram_tensor("a1h", [128, 3, 2, bc], FP8, kind="ExternalInput")
    w1 = nc.dram_tensor("w1dr", [32, 128, 3, 2, 128], FP8, kind="ExternalInput")
    wS2 = nc.dram_tensor("wS2", [16, 4, 128, 7, 2, 2, 128], FP8,
                         kind="ExternalInput")
    wS3 = nc.dram_tensor("wS3", [16, 4, 128, 7, 2, 2, 128], FP8,
                         kind="ExternalInput")
    w4 = nc.dram_tensor("w4dr", [128, 16, 2, 16], FP8, kind="ExternalInput")
    thrs = nc.dram_tensor("thrs", [128, 3, 32], FP32, kind="ExternalInput")
    c4 = nc.dram_tensor("c4", [16, 2], FP32, kind="ExternalInput")
    out = nc.dram_tensor("out", [128, bc // 128, OUT], FP32,
                         kind="ExternalOutput")

    with tile.TileContext(nc) as tc, ExitStack() as ctx:
        consts = ctx.enter_context(tc.tile_pool(name="consts", bufs=1))
        xpool = ctx.enter_context(tc.tile_pool(name="xpool", bufs=2))
        apool = ctx.enter_context(tc.tile_pool(name="apool", bufs=2))
        wpool = ctx.enter_context(tc.tile_pool(name="wpool", bufs=2))
        wspool = ctx.enter_context(tc.tile_pool(name="wspool", bufs=4))
        tpool = ctx.enter_context(tc.tile_pool(name="tpool", bufs=4))
        spool = ctx.enter_context(tc.tile_pool(name="spool", bufs=3))
        smpool = ctx.enter_context(tc.tile_pool(name="smpool", bufs=3))
        psum_pool = ctx.enter_context(
            tc.tile_pool(name="psum", bufs=8, space="PSUM"))

        thrs_sb = consts.tile([128, 3, 32], FP32, tag="thrs")
        c4_sb = consts.tile([16, 2], FP32, tag="c4")
        w4_sb = consts.tile([128, 16, 2, 16], FP8, tag="w4")
        ident = consts.tile([128, 128], FP32, tag="ident")
        h4 = consts.tile([16, bc], FP32, tag="h4")
        out_sb = consts.tile([128, BT, OUT], FP32, tag="outsb")
        thr1_sb = thrs_sb[:, 0, :]
        thr2_sb = thrs_sb[:, 1, :]
        thr3_sb = thrs_sb[:, 2, :]

        # ---- act1 = [x >= 0] {0,1} fp8 packed host-side, DMA'd direct.
        # act1 lives in the big act ring (slot0); act3 reuses the slot later
        # (its first write is ordered after act1's last read by Tile deps).
        act1big = apool.tile([128, 16, 2, bc], FP8, tag="actbig",
                             name="act1")
        act1 = act1big[:, 0:3, :, :]
        nc.sync.dma_start(out=act1[:, 0, :, :], in_=a1h[:, 0, :, :])
        nc.scalar.dma_start(out=act1[:, 1:3, :, :], in_=a1h[:, 1:3, :, :])

        # consts follow x on the rings; their data is needed much later
        nc.scalar.dma_start(out=thrs_sb[:], in_=thrs[:])
        nc.scalar.dma_start(out=c4_sb[:], in_=c4[:])
        nc.scalar.dma_start(out=w4_sb[:], in_=w4[:])
        make_identity(nc, ident[:])

        # PE warm-up: spans the act1/w1 DMA wait so layer 1 opens at speed.
        warm = consts.tile([128, 2, 128], FP8, tag="warm")
        nc.gpsimd.memset(warm[:], 0.0)
        wps = psum_pool.tile([128, 512], FP32, tag="psum", name="warmps")
        for _ in range(24):
            nc.tensor.matmul(
                wps[:, 0:128], lhsT=warm[:, :, 0:128], rhs=warm[:],
                start=True, stop=True, perf_mode=DR)

        # ---- layer 1 (768 -> 4096), classic DR, in TWO n-phases so that
        # the batch columns Strassen block 0 needs ({0:512, 1024:1536}) are
        # complete halfway through -> GpSimd builds L2's T-combos during
        # phase B and layer 2 opens with zero stall.
        act2 = apool.tile([128, 16, 2, bc], FP8, tag="actbig")
        t2_b0 = None
        for phase in range(2):
            for mt in range(32):
                wt = wpool.tile([128, 3, 2, 128], FP8, tag="w")
                nc.sync.dma_start(out=wt[:], in_=w1[mt])
                pss = [psum_pool.tile([128, 512], FP32, tag="psum",
                                      name=f"l1ps{phase}_{mt}_{n}")
                       for n in range(2)]
                for i, n in enumerate((phase, phase + 2)):
                    for c in range(3):
                        nc.tensor.matmul(
                            pss[i][:],
                            lhsT=wt[:, c, :, :],
                            rhs=act1[:, c, :, 512 * n:512 * (n + 1)],
                            start=(c == 0), stop=(c == 2), perf_mode=DR)
                for i, n in enumerate((phase, phase + 2)):
                    nc.vector.tensor_scalar(
                        out=act2[:, mt // 2, mt % 2, 512 * n:512 * (n + 1)],
                        in0=pss[i][:], scalar1=thr1_sb[:, mt:mt + 1],
                        scalar2=None, op0=ALU.is_ge)
            if phase == 0:
                t2_b0 = _emit_t12(nc, tpool, act2, 0, "L2")
                t2_b0.append(_emit_t3(nc, tpool, act2, 0, "L2"))
                t2_b0.append(_emit_t4h(nc, tpool, act2, 0, t2_b0[1], 0, "L2"))
                t2_b0.append(_emit_t4h(nc, tpool, act2, 0, t2_b0[1], 1, "L2"))

        # ---- layers 2+3: Strassen-Winograd ----
        # T-combos are emitted AHEAD of use: the gpsimd pair (t1,t2) a whole
        # block early into fresh pool slots, the DVE pair (t3,t4) mid-way
        # through the previous block where DVE has slack.
        act3 = apool.tile([128, 16, 2, bc], FP8, tag="actbig")
        act4 = apool.tile([128, 16, 2, bc], FP8, tag="actbig")
        t2t = [t2_b0, None]
        t3t = [None, None]

        def grow(tl, idx, val):
            tl[idx] = val if tl[idx] is None else tl[idx] + val

        def mk_hooks(tl, idx, act, blk, nm):
            return {
                (8,): lambda: grow(tl, idx, _emit_t12(nc, tpool, act, blk, nm)),
                (10,): lambda: grow(tl, idx, [_emit_t3(nc, tpool, act, blk, nm)]),
                (12,): lambda: grow(tl, idx, [_emit_t4h(nc, tpool, act, blk,
                                                        tl[idx][1], 0, nm)]),
                (14,): lambda: grow(tl, idx, [_emit_t4h(nc, tpool, act, blk,
                                                        tl[idx][1], 1, nm)]),
            }

        def at(blk, h):
            return {(blk, r[0]): f for r, f in h.items()}

        hooks_l2 = {}
        hooks_l2.update(at(0, mk_hooks(t2t, 1, act2, 1, "L2")))
        hooks_l2.update(at(1, mk_hooks(t3t, 0, act3, 0, "L3")))
        hooks_l3 = at(0, mk_hooks(t3t, 1, act3, 1, "L3"))
        _strassen_layer(nc, wspool, tpool, spool, psum_pool, act2, wS2,
                        thr2_sb, act3, bc, t2t, hooks=hooks_l2)
        _strassen_layer(nc, wspool, tpool, spool, psum_pool, act3, wS3,
                        thr3_sb, act4, bc, t3t, hooks=hooks_l3)

        # ---- layer 4: logits (M padded 10->16), affine folds BN+rowsum.
        sh = smpool.tile([128, BT, OUT], FP32, tag="sh", bufs=1)
        se = smpool.tile([128, BT], FP32, tag="se", bufs=1)
        ls = smpool.tile([128, BT], FP32, tag="ls", bufs=1)

        def _l4_softmax_head(g):
            # transpose group g's batch tiles + max/shift on DVE; runs one
            # n-group behind the L4 matmuls so the PE never stalls on it
            for bt in range(4 * g, 4 * g + 4):
                tp = psum_pool.tile([128, OUT], FP32, tag="psum",
                                    name=f"tp{bt}")
                nc.tensor.transpose(
                    tp[:], h4[0:OUT, 128 * bt:128 * (bt + 1)],
                    ident[0:OUT, 0:OUT])
                mx = smpool.tile([128, 1], FP32, tag="mx", name=f"mx{bt}")
                nc.vector.reduce_max(mx[:], tp[:], axis=mybir.AxisListType.X)
                nc.vector.tensor_scalar(
                    out=sh[:, bt, :], in0=tp[:], scalar1=mx[:], scalar2=None,
                    op0=ALU.subtract)
        for n in range(NT):
            ps4 = psum_pool.tile([16, 512], FP32, tag="psum", name=f"ps4_{n}")
            for c in range(16):
                nc.tensor.matmul(
                    ps4[:],
                    lhsT=w4_sb[:, c, :, :],
                    rhs=act4[:, c, :, 512 * n:512 * (n + 1)],
                    start=(c == 0),
                    stop=(c == 15),
                    perf_mode=DR,
                )
            # affine on DVE: out = in*scale + bias
            nc.vector.tensor_scalar(
                out=h4[:, 512 * n:512 * (n + 1)], in0=ps4[:],
                scalar1=c4_sb[:, 0:1], scalar2=c4_sb[:, 1:2],
                op0=ALU.mult, op1=ALU.add,
            )
        for g in range(NT):
            _l4_softmax_head(g)
        ex = smpool.tile([128, BT, OUT], FP32, tag="ex", bufs=1)
        for bt in range(BT):  # all Exp together: one ACT table load
            nc.scalar.activation(
                ex[:, bt, :], sh[:, bt, :], mybir.ActivationFunctionType.Exp,
                accum_out=se[:, bt:bt + 1])
        nc.scalar.activation(  # single Ln over all batch tiles
            ls[:], se[:], mybir.ActivationFunctionType.Ln)
        for bt in range(BT):
            nc.vector.tensor_scalar(
                out=out_sb[:, bt, :], in0=sh[:, bt, :],
                scalar1=ls[:, bt:bt + 1], scalar2=None,
                op0=ALU.subtract)

        # out dram is [128, BT, OUT] (partition-major, fully contiguous DMA);
        # the host reassembles batch order with a free transpose.
        nc.sync.dma_start(out=out[:], in_=out_sb[:])

    nc.compile()
    return nc


# --------------------------------------------------------------------------
# Host-side preparation
# --------------------------------------------------------------------------

def _pack_w_dr(ws_t):
    """[Fin, Fout] {-1,+1} -> [Mt, 128, C, 2, 128] fp8 DoubleRow layout.

    wdr[mt, ki, c, ko, mi] = ws_t[256*c + 128*ko + ki, 128*mt + mi]
    """
    fin, fout = ws_t.shape
    C, Mt = fin // 256, fout // 128
    w = ws_t.reshape(C, 2, 128, Mt, 128).transpose(3, 2, 0, 1, 4)
    return np.ascontiguousarray(w).astype(NP_FP8)


def _pack_strassen(ws):
    """[4096 out, 4096 in] +-1 -> [16, 128, 7, 8, 2, 128] fp8: the 7
    Winograd left-operand combos of W' = 2*ws in DoubleRow layout,
    product order [P1, P2n, P3, P5n, P6, P7, P4] (P2, P5 negated so the
    device combines become single is_ge scalar_tensor_tensor ops)."""
    Wp = 2.0 * ws
    W11, W12 = Wp[:2048, :2048], Wp[:2048, 2048:]
    W21, W22 = Wp[2048:, :2048], Wp[2048:, 2048:]
    S1 = W21 + W22
    S2 = S1 - W11
    S3 = W11 - W21
    S4 = W12 - S2
    Ls = [W11, -W12, S4, -S1, S2, S3, W22]
    # L[128r+mi, 256c+128i+ki] -> arr[r, p, c, ki, i, mi] -> [r, ki, p, c, i, mi]
    arr = np.stack(
        [L.reshape(16, 128, 8, 2, 128).transpose(0, 2, 4, 3, 1) for L in Ls],
        axis=1)  # [r, p, c, ki, i, mi]
    arr = arr.reshape(16, 7, 4, 2, 128, 2, 128).transpose(0, 2, 4, 1, 3, 5, 6)
    return np.ascontiguousarray(arr).astype(NP_FP8)  # [r, q, ki, p, cc, i, mi]


def prepare_consts(inputs):
    """Fold sign(w), BN, bias and the 0/1-activation rowsum correction.

    The device computes, per layer, a_dev = [mmA~ >= thr] where
    mmA~ = W~sign @ a_dev_prev over {0,1} activations. Negative BN scales
    (alpha <= 0) are handled exactly by tracking a per-neuron flip bit
    (a_true = 1 - a_dev) that folds into the *next* layer's weight signs:
    with s~ = s * (1-2*flip_in), mm_full = 2*(s~ @ a_dev) - rowsum(s~)
    holds for any flip pattern. Thresholds use integer snapping (mmA is
    always an integer), making the device comparison tie-free/exact.
    Layers 2+3 carry weights W' = 2*W (Strassen packing), so their
    thresholds are doubled (still integers -> still tie-free).
    """
    consts = {}
    flip_in = np.zeros(IND)  # input layer: a_dev = [x >= 0] = ste_sign, exact
    for i in (1, 2, 3, 4):
        w = np.asarray(inputs[f"w{i}"]).astype(np.float64)
        b = np.asarray(inputs[f"b{i}"]).astype(np.float64)
        g = np.asarray(inputs[f"g{i}"]).astype(np.float64)
        be = np.asarray(inputs[f"be{i}"]).astype(np.float64)
        m = np.asarray(inputs[f"m{i}"]).astype(np.float64)
        v = np.asarray(inputs[f"v{i}"]).astype(np.float64)
        ws = np.where(w >= 0, 1.0, -1.0) * (1.0 - 2.0 * flip_in)  # [fo, fi]
        rowsum = ws.sum(axis=1)                                   # [fo]
        alpha = g / np.sqrt(v + EPS)
        if i < 4:
            # BN(mm_full + b) >= 0 with mm_full = 2*mmA - rowsum:
            #   alpha > 0:  a_true = [mmA >= u],  u = (m-b-be/a+rowsum)/2
            #   alpha < 0:  a_true = [mmA <= u] = 1 - [mmA >= floor(u)+1]
            #   alpha == 0: BN = be, constant sign
            u = (m - b - be / alpha_safe(alpha) + rowsum) / 2.0
            pos = alpha > 0
            thr = np.where(pos, np.ceil(u), np.floor(u) + 1.0)
            zero = alpha == 0
            if zero.any():
                # constant: a_true = [be >= 0]; force a_dev accordingly
                thr = np.where(zero & (be >= 0), -1e30, thr)
                thr = np.where(zero & (be < 0), 1e30, thr)
                pos = pos | zero
            flip_in = (~pos).astype(np.float64)
            scale = 1.0 if i == 1 else 2.0  # Strassen layers carry 2x weights
            consts.setdefault("_thrs", []).append(
                (scale * thr).reshape(32, 128).T.astype(np.float32))
            if i == 1:
                consts["w1dr"] = _pack_w_dr(ws.T)
            else:
                consts[f"wS{i}"] = _pack_strassen(ws)
        else:
            # logits = mmA*(2*alpha) + ((b - m - rowsum)*alpha + be), pad to 16
            scale = 2.0 * alpha
            beta = (b - m - rowsum) * alpha + be
            c4 = np.zeros((16, 2), np.float32)
            c4[:10, 0] = scale.astype(np.float32)
            c4[:10, 1] = beta.astype(np.float32)
            consts["c4"] = c4
            ws_t_pad = np.zeros((HID, 16), np.float64)
            ws_t_pad[:, :10] = ws.T
            # w4dr[ki, c, ko, m] = ws_t_pad[256*c + 128*ko + ki, m]
            w4 = ws_t_pad.reshape(16, 2, 128, 16).transpose(2, 0, 1, 3)
            consts["w4dr"] = np.ascontiguousarray(w4).astype(NP_FP8)
    consts["thrs"] = np.ascontiguousarray(
        np.stack(consts.pop("_thrs"), axis=1))  # [128, 3, 32]
    return consts


def alpha_safe(a):
    return np.where(a == 0, 1.0, a)


_PROG_CACHE = {}


def _get_program(bc=BC):
    if bc not in _PROG_CACHE:
        _PROG_CACHE[bc] = build_program(bc)
    return _PROG_CACHE[bc]


def kernel(**inputs):
    global LAST_RESULTS
    x = np.asarray(inputs["x"], np.float32)
    assert x.shape == (B, 784)
    consts = prepare_consts(inputs)
    # act1 = [x >= 0] in {0,1} fp8 packed host-side straight into the
    # feature-major device layout a1h[p, c, i, n] = a[256c+128i+p, n].
    a1_full = (x[:, :IND].T >= 0).astype(NP_FP8)  # [768, B]
    a1_full = a1_full.reshape(3, 2, 128, B).transpose(2, 0, 1, 3)

    nc = _get_program(BC)
    in_maps = []
    for c in range(N_CORES):
        m = {"a1h": np.ascontiguousarray(a1_full[:, :, :, c * BC:(c + 1) * BC])}
        m.update(consts)
        in_maps.append(m)

    res = run_bass_kernel_spmd(
        nc, in_maps, core_ids=list(range(N_CORES)), trace=TRACE,
        **TRACE_KWARGS)
    LAST_RESULTS = res
    # device out is [128, BT, 10] partition-major; restore batch order
    outs = [np.ascontiguousarray(r["out"].transpose(1, 0, 2).reshape(BC, OUT))
            for r in res.results]
    return np.concatenate(outs, axis=0)


# revision 20
# speedup vs baseline: 1.2495x; 1.0184x over previous
"""Binarized-MLP (BinaryNet) forward on 8 Trainium2 NeuronCores.

Reference computation (per nn_FC_large):
    h = sign(x[:, :768]) @ sign(w1).T + b1 ; BN1 ; -> sign
    h = sign(h) @ sign(w2).T + b2         ; BN2 ; -> sign
    h = sign(h) @ sign(w3).T + b3         ; BN3 ; -> sign
    h = sign(h) @ sign(w4).T + b4         ; BN4 ; log_softmax

Strategy (data parallel, batch 16384 -> 2048 rows/core):
  * Activations are a in {0,1} fp8 (a = [pre-act >= 0]); the identity
    sign_mm = 2*(Wsign @ a) - rowsum(Wsign) folds the rowsum into
    per-neuron integer thresholds, so every epilogue is one DVE op.
  * Matmuls run in fp8e4 with perf_mode=DoubleRow (K=256/instruction).
  * Layers 2+3 (4096x4096) use one level of Strassen-Winograd: 7 products
    of half-size blocks instead of 8 (PE work x7/8). Weight-side combos
    (host, values in {-8..8}) and activation-side combos T1..T4 (GpSimd,
    values in {-2..2}) are exact in fp8; products are integers well inside
    fp32, so the whole pipeline stays bit-exact w.r.t. the reference
    (weights carry a 2x so thresholds are simply doubled).
    The 7 output-side adds fold thresholds + compare into DVE
    scalar_tensor_tensor ops: e.g. C11 = [(P1 - t) >= (-P2)] with the
    (-P2) product negated host-side.
  * Layer-4 logits are PE-transposed to batch-major; log_softmax runs
    on-device (DVE/ACT) with Exp batched to avoid ACT table thrash.

Everything is hardcoded for x:[16384,784], layers 768->4096->4096->4096->10.
"""

import numpy as np
import ml_dtypes
from contextlib import ExitStack

import concourse.mybir as mybir
import concourse.tile as tile
from concourse import bacc
from concourse.bass_utils import run_bass_kernel_spmd
from concourse.masks import make_identity

FP32 = mybir.dt.float32
BF16 = mybir.dt.bfloat16
FP8 = mybir.dt.float8e4
NP_FP8 = ml_dtypes.float8_e4m3
NP_BF16 = ml_dtypes.bfloat16

EPS = 1e-5
B, IND, HID, OUT = 16384, 768, 4096, 10
N_CORES = 8
BC = B // N_CORES  # 2048 batch rows per core

# Knobs (test.py may flip TRACE before calling kernel()).
TRACE = False
TRACE_KWARGS = {}
LAST_RESULTS = None  # BassKernelResults of the most recent run

ALU = mybir.AluOpType


# --------------------------------------------------------------------------
# Device program
# --------------------------------------------------------------------------

def _emit_t12(nc, tpool, act_in, blk, lname):
    """t1 = B12-B11, t2 = B22-t1 on GpSimd (slow but emitted a whole block
    ahead into fresh pool slots, so it never gates the PE)."""
    nl = slice(512 * blk, 512 * blk + 512)
    nr = slice(1024 + 512 * blk, 1536 + 512 * blk)
    t1 = tpool.tile([128, 8, 2, 512], FP8, tag="t", bufs=5,
                    name=f"t1{lname}{blk}")
    t2 = tpool.tile([128, 8, 2, 512], FP8, tag="t", bufs=5,
                    name=f"t2{lname}{blk}")
    g = nc.gpsimd
    g.tensor_tensor(out=t1[:], in0=act_in[:, 0:8, :, nr],
                    in1=act_in[:, 0:8, :, nl], op=ALU.subtract)
    g.tensor_tensor(out=t2[:], in0=act_in[:, 8:16, :, nr], in1=t1[:],
                    op=ALU.subtract)
    return [t1, t2]


def _emit_t3(nc, tpool, act_in, blk, lname):
    """t3 = B22-B12 on DVE (fast; emitted mid-previous-block)."""
    nl = slice(512 * blk, 512 * blk + 512)
    nr = slice(1024 + 512 * blk, 1536 + 512 * blk)
    t3 = tpool.tile([128, 8, 2, 512], FP8, tag="t", bufs=5,
                    name=f"t3{lname}{blk}")
    nc.vector.tensor_tensor(out=t3[:], in0=act_in[:, 8:16, :, nr],
                            in1=act_in[:, 0:8, :, nr], op=ALU.subtract)
    return t3


def _emit_t4h(nc, tpool, act_in, blk, t2, half, lname):
    """One k-half of t4 = t2 - B21 on DVE (own small pool ring)."""
    nl = slice(512 * blk, 512 * blk + 512)
    cs = slice(4 * half, 4 * half + 4)
    t4 = tpool.tile([128, 4, 2, 512], FP8, tag="t4", bufs=3,
                    name=f"t4{lname}{blk}h{half}")
    nc.vector.tensor_tensor(out=t4[:], in0=t2[:, cs, :, :],
                            in1=act_in[:, 8 + 4 * half:12 + 4 * half, :, nl],
                            op=ALU.subtract)
    return t4


def _strassen_layer(nc, wpool, tpool, spool, psum_pool, act_in, wS, thr_sb,
                    act_out, bc, t_tiles, hooks=None):
    """One 4096x4096 binarized layer via 1-level Strassen-Winograd.

    act_in : SBUF AP [128, 16, 2, bc] fp8 {0,1}
    wS     : DRAM [16, 2, 128, 7, 4, 2, 128] fp8 (7 left-combos, DR layout)
    thr_sb : SBUF [128, 32] fp32 (2x integer-snapped thresholds)
    act_out: SBUF AP [128, 16, 2, bc] fp8
    t_tiles: [t-list for blk0, t-list for blk1] (emitted ahead)
    hooks  : dict (blk, r) -> callback() emitting a future block's T-combos
             at a point where its inputs are ready and DVE/GpSimd have slack.
    """
    DR = mybir.MatmulPerfMode.DoubleRow
    for blk in range(2):
        nl = slice(512 * blk, 512 * blk + 512)
        nr = slice(1024 + 512 * blk, 1536 + 512 * blk)
        t1, t2, t3, t4a, t4b = t_tiles[blk]  # filled ahead by hooks
        # rhs for products [P1, P2n, P3, P5n, P6, P7, P4]
        rhs = [lambda c: act_in[:, c, :, nl],
               lambda c: act_in[:, 8 + c, :, nl],
               lambda c: act_in[:, 8 + c, :, nr],
               lambda c: t1[:, c, :, :],
               lambda c: t2[:, c, :, :],
               lambda c: t3[:, c, :, :],
               lambda c: (t4a if c < 4 else t4b)[:, c % 4, :, :]]
        for r in range(16):
            t_top = thr_sb[:, r:r + 1]
            t_bot = thr_sb[:, 16 + r:17 + r]
            o_tl = act_out[:, r // 2, r % 2, nl]       # C11
            o_tr = act_out[:, r // 2, r % 2, nr]       # C12
            o_bl = act_out[:, 8 + r // 2, r % 2, nl]   # C21
            o_br = act_out[:, 8 + r // 2, r % 2, nr]   # C22
            pb = [psum_pool.tile([128, 512], FP32, tag="psum",
                                 name=f"sp{blk}_{r}_{p}") for p in range(7)]
            s0 = spool.tile([128, 512], FP32, tag="v", name=f"s0_{blk}{r}")
            v2 = spool.tile([128, 512], FP32, tag="v", name=f"v2_{blk}{r}")
            v3 = spool.tile([128, 512], FP32, tag="v", name=f"v3_{blk}{r}")
            v4 = spool.tile([128, 512], FP32, tag="v", name=f"v4_{blk}{r}")
            for q in range(4):
                wt = wpool.tile([128, 7, 2, 2, 128], FP8, tag="ws")
                nc.sync.dma_start(out=wt[:], in_=wS[r, q])
                for p in range(7):
                    for cc in range(2):
                        c = 2 * q + cc
                        nc.tensor.matmul(
                            pb[p][:], lhsT=wt[:, p, cc, :, :], rhs=rhs[p](c),
                            start=(c == 0), stop=(c == 7), perf_mode=DR)
            # DVE reads at most one PSUM operand (the STT in1 port), so
            # P1 is copied to SBUF once and every combine chains off SBUF.
            nc.vector.tensor_scalar(               # s0 = P1
                out=s0[:], in0=pb[0][:], scalar1=0.0, scalar2=None,
                op0=ALU.add)
            nc.vector.scalar_tensor_tensor(        # C11 = [(P1 - t) >= -P2]
                out=o_tl, in0=s0[:], scalar=t_top, in1=pb[1][:],
                op0=ALU.subtract, op1=ALU.is_ge)
            nc.vector.scalar_tensor_tensor(        # V2 = U2 = P1 + P6
                out=v2[:], in0=s0[:], scalar=0.0, in1=pb[4][:],
                op0=ALU.bypass, op1=ALU.add)
            nc.vector.scalar_tensor_tensor(        # V3 = U2 + P7 - t_bot
                out=v3[:], in0=v2[:], scalar=t_bot, in1=pb[5][:],
                op0=ALU.subtract, op1=ALU.add)
            nc.vector.scalar_tensor_tensor(        # V4 = U2 + P3 - t_top
                out=v4[:], in0=v2[:], scalar=t_top, in1=pb[2][:],
                op0=ALU.subtract, op1=ALU.add)
            nc.vector.scalar_tensor_tensor(        # C12 = [V4 >= -P5]
                out=o_tr, in0=v4[:], scalar=0.0, in1=pb[3][:],
                op0=ALU.bypass, op1=ALU.is_ge)
            nc.vector.scalar_tensor_tensor(        # C21 = [V3 >= P4]
                out=o_bl, in0=v3[:], scalar=0.0, in1=pb[6][:],
                op0=ALU.bypass, op1=ALU.is_ge)
            nc.vector.scalar_tensor_tensor(        # C22 = [V3 >= -P5]
                out=o_br, in0=v3[:], scalar=0.0, in1=pb[3][:],
                op0=ALU.bypass, op1=ALU.is_ge)
            if hooks and (blk, r) in hooks:
                hooks[(blk, r)]()


def build_program(bc=BC):
    """Build the per-core Bass/Tile program (SPMD; identical on all cores)."""
    NT = bc // 512
    BT = bc // 128
    DR = mybir.MatmulPerfMode.DoubleRow

    nc = bacc.Bacc(None, target_bir_lowering=False, debug=False)

    a1h = nc.dram_tensor("a1h", [128, 3, 2, bc], FP8, kind="ExternalInput")
    w1 = nc.dram_tensor("w1dr", [32, 128, 3, 2, 128], FP8, kind="ExternalInput")
    wS2 = nc.dram_tensor("wS2", [16, 4, 128, 7, 2, 2, 128], FP8,
                         kind="ExternalInput")
    wS3 = nc.dram_tensor("wS3", [16, 4, 128, 7, 2, 2, 128], FP8,
                         kind="ExternalInput")
    w4 = nc.dram_tensor("w4dr", [128, 16, 2, 16], FP8, kind="ExternalInput")
    thrs = nc.dram_tensor("thrs", [128, 3, 32], FP32, kind="ExternalInput")
    c4 = nc.dram_tensor("c4", [16, 2], FP32, kind="ExternalInput")
    out = nc.dram_tensor("out", [128, bc // 128, OUT], FP32,
                         kind="ExternalOutput")

    with tile.TileContext(nc) as tc, ExitStack() as ctx:
        consts = ctx.enter_context(tc.tile_pool(name="consts", bufs=1))
        xpool = ctx.enter_context(tc.tile_pool(name="xpool", bufs=2))
        apool = ctx.enter_context(tc.tile_pool(name="apool", bufs=2))
        wpool = ctx.enter_context(tc.tile_pool(name="wpool", bufs=2))
        wspool = ctx.enter_context(tc.tile_pool(name="wspool", bufs=3))
        tpool = ctx.enter_context(tc.tile_pool(name="tpool", bufs=4))
        spool = ctx.enter_context(tc.tile_pool(name="spool", bufs=3))
        smpool = ctx.enter_context(tc.tile_pool(name="smpool", bufs=3))
        psum_pool = ctx.enter_context(
            tc.tile_pool(name="psum", bufs=8, space="PSUM"))

        thrs_sb = consts.tile([128, 3, 32], FP32, tag="thrs")
        c4_sb = consts.tile([16, 2], FP32, tag="c4")
        w4_sb = consts.tile([128, 16, 2, 16], FP8, tag="w4")
        ident = consts.tile([128, 128], FP32, tag="ident")
        identb = consts.tile([128, 128], BF16, tag="identb")
        h4 = consts.tile([16, bc], BF16, tag="h4")
        out_sb = consts.tile([128, BT, OUT], FP32, tag="outsb")
        thr1_sb = thrs_sb[:, 0, :]
        thr2_sb = thrs_sb[:, 1, :]
        thr3_sb = thrs_sb[:, 2, :]

        # ---- act1 = [x >= 0] {0,1} fp8 packed host-side, DMA'd direct.
        # act1 lives in the big act ring (slot0); act3 reuses the slot later
        # (its first write is ordered after act1's last read by Tile deps).
        act1big = apool.tile([128, 16, 2, bc], FP8, tag="actbig",
                             name="act1")
        act1 = act1big[:, 0:3, :, :]
        nc.sync.dma_start(out=act1[:, 0, :, :], in_=a1h[:, 0, :, :])
        nc.scalar.dma_start(out=act1[:, 1:3, :, :], in_=a1h[:, 1:3, :, :])

        # consts follow x on the rings; their data is needed much later
        nc.scalar.dma_start(out=thrs_sb[:], in_=thrs[:])
        nc.scalar.dma_start(out=c4_sb[:], in_=c4[:])
        nc.scalar.dma_start(out=w4_sb[:], in_=w4[:])
        make_identity(nc, ident[:])
        nc.vector.tensor_scalar(out=identb[:], in0=ident[:], scalar1=0.0,
                                scalar2=None, op0=ALU.add)

        # PE warm-up: spans the act1/w1 DMA wait so layer 1 opens at speed.
        warm = consts.tile([128, 2, 128], FP8, tag="warm")
        nc.gpsimd.memset(warm[:], 0.0)
        wps = psum_pool.tile([128, 512], FP32, tag="psum", name="warmps")
        for _ in range(24):
            nc.tensor.matmul(
                wps[:, 0:128], lhsT=warm[:, :, 0:128], rhs=warm[:],
                start=True, stop=True, perf_mode=DR)

        # ---- layer 1 (768 -> 4096), classic DR, in TWO n-phases so that
        # the batch columns Strassen block 0 needs ({0:512, 1024:1536}) are
        # complete halfway through -> GpSimd builds L2's T-combos during
        # phase B and layer 2 opens with zero stall.
        act2 = apool.tile([128, 16, 2, bc], FP8, tag="actbig")
        t2_b0 = None
        for phase in range(2):
            for mt in range(32):
                wt = wpool.tile([128, 3, 2, 128], FP8, tag="w")
                nc.sync.dma_start(out=wt[:], in_=w1[mt])
                pss = [psum_pool.tile([128, 512], FP32, tag="psum",
                                      name=f"l1ps{phase}_{mt}_{n}")
                       for n in range(2)]
                for i, n in enumerate((phase, phase + 2)):
                    for c in range(3):
                        nc.tensor.matmul(
                            pss[i][:],
                            lhsT=wt[:, c, :, :],
                            rhs=act1[:, c, :, 512 * n:512 * (n + 1)],
                            start=(c == 0), stop=(c == 2), perf_mode=DR)
                for i, n in enumerate((phase, phase + 2)):
                    nc.vector.tensor_scalar(
                        out=act2[:, mt // 2, mt % 2, 512 * n:512 * (n + 1)],
                        in0=pss[i][:], scalar1=thr1_sb[:, mt:mt + 1],
                        scalar2=None, op0=ALU.is_ge)
            if phase == 0:
                t2_b0 = _emit_t12(nc, tpool, act2, 0, "L2")
                t2_b0.append(_emit_t3(nc, tpool, act2, 0, "L2"))
                t2_b0.append(_emit_t4h(nc, tpool, act2, 0, t2_b0[1], 0, "L2"))
                t2_b0.append(_emit_t4h(nc, tpool, act2, 0, t2_b0[1], 1, "L2"))

        # ---- layers 2+3: Strassen-Winograd ----
        # T-combos are emitted AHEAD of use: the gpsimd pair (t1,t2) a whole
        # block early into fresh pool slots, the DVE pair (t3,t4) mid-way
        # through the previous block where DVE has slack.
        act3 = apool.tile([128, 16, 2, bc], FP8, tag="actbig")
        act4 = apool.tile([128, 16, 2, bc], FP8, tag="actbig")
        t2t = [t2_b0, None]
        t3t = [None, None]

        def grow(tl, idx, val):
            tl[idx] = val if tl[idx] is None else tl[idx] + val

        def mk_hooks(tl, idx, act, blk, nm):
            def late():
                grow(tl, idx, [_emit_t3(nc, tpool, act, blk, nm)])
                grow(tl, idx, [_emit_t4h(nc, tpool, act, blk,
                                         tl[idx][1], 0, nm)])
                grow(tl, idx, [_emit_t4h(nc, tpool, act, blk,
                                         tl[idx][1], 1, nm)])
            return {
                (8,): lambda: grow(tl, idx, _emit_t12(nc, tpool, act, blk, nm)),
                (15,): late,
            }

        def at(blk, h):
            return {(blk, r[0]): f for r, f in h.items()}

        hooks_l2 = {}
        hooks_l2.update(at(0, mk_hooks(t2t, 1, act2, 1, "L2")))
        hooks_l2.update(at(1, mk_hooks(t3t, 0, act3, 0, "L3")))
        hooks_l3 = at(0, mk_hooks(t3t, 1, act3, 1, "L3"))
        _strassen_layer(nc, wspool, tpool, spool, psum_pool, act2, wS2,
                        thr2_sb, act3, bc, t2t, hooks=hooks_l2)
        _strassen_layer(nc, wspool, tpool, spool, psum_pool, act3, wS3,
                        thr3_sb, act4, bc, t3t, hooks=hooks_l3)

        # ---- layer 4: logits (M padded 10->16), affine folds BN+rowsum.
        sh = smpool.tile([128, BT, OUT], FP32, tag="sh", bufs=1)
        se = smpool.tile([128, BT], FP32, tag="se", bufs=1)
        ls = smpool.tile([128, BT], FP32, tag="ls", bufs=1)

        def _l4_softmax_head(g):
            # transpose group g's batch tiles + max/shift on DVE; runs one
            # n-group behind the L4 matmuls so the PE never stalls on it
            for bt in range(4 * g, 4 * g + 4):
                tp = psum_pool.tile([128, OUT], BF16, tag="psum",
                                    name=f"tp{bt}")
                nc.tensor.transpose(
                    tp[:], h4[0:OUT, 128 * bt:128 * (bt + 1)],
                    identb[0:OUT, 0:OUT])
                mx = smpool.tile([128, 1], FP32, tag="mx", name=f"mx{bt}")
                nc.vector.reduce_max(mx[:], tp[:], axis=mybir.AxisListType.X)
                nc.vector.tensor_scalar(
                    out=sh[:, bt, :], in0=tp[:], scalar1=mx[:], scalar2=None,
                    op0=ALU.subtract)
        for n in range(NT):
            ps4 = psum_pool.tile([16, 512], FP32, tag="psum", name=f"ps4_{n}")
            for c in range(16):
                nc.tensor.matmul(
                    ps4[:],
                    lhsT=w4_sb[:, c, :, :],
                    rhs=act4[:, c, :, 512 * n:512 * (n + 1)],
                    start=(c == 0),
                    stop=(c == 15),
                    perf_mode=DR,
                )
            # affine on DVE: out = in*scale + bias
            nc.vector.tensor_scalar(
                out=h4[:, 512 * n:512 * (n + 1)], in0=ps4[:],
                scalar1=c4_sb[:, 0:1], scalar2=c4_sb[:, 1:2],
                op0=ALU.mult, op1=ALU.add,
            )
        for g in range(NT):
            _l4_softmax_head(g)
        ex = smpool.tile([128, BT, OUT], FP32, tag="ex", bufs=1)
        for bt in range(BT):  # all Exp together: one ACT table load
            nc.scalar.activation(
                ex[:, bt, :], sh[:, bt, :], mybir.ActivationFunctionType.Exp,
                accum_out=se[:, bt:bt + 1])
        nc.scalar.activation(  # single Ln over all batch tiles
            ls[:], se[:], mybir.ActivationFunctionType.Ln)
        for bt in range(BT):
            nc.vector.tensor_scalar(
                out=out_sb[:, bt, :], in0=sh[:, bt, :],
                scalar1=ls[:, bt:bt + 1], scalar2=None,
                op0=ALU.subtract)

        # out dram is [128, BT, OUT] (partition-major, fully contiguous DMA);
        # the host reassembles batch order with a free transpose.
        nc.sync.dma_start(out=out[:], in_=out_sb[:])

    nc.compile()
    return nc


# --------------------------------------------------------------------------
# Host-side preparation
# --------------------------------------------------------------------------

def _pack_w_dr(ws_t):
    """[Fin, Fout] {-1,+1} -> [Mt, 128, C, 2, 128] fp8 DoubleRow layout.

    wdr[mt, ki, c, ko, mi] = ws_t[256*c + 128*ko + ki, 128*mt + mi]
    """
    fin, fout = ws_t.shape
    C, Mt = fin // 256, fout // 128
    w = ws_t.reshape(C, 2, 128, Mt, 128).transpose(3, 2, 0, 1, 4)
    return np.ascontiguousarray(w).astype(NP_FP8)


def _pack_strassen(ws):
    """[4096 out, 4096 in] +-1 -> [16, 128, 7, 8, 2, 128] fp8: the 7
    Winograd left-operand combos of W' = 2*ws in DoubleRow layout,
    product order [P1, P2n, P3, P5n, P6, P7, P4] (P2, P5 negated so the
    device combines become single is_ge scalar_tensor_tensor ops)."""
    Wp = 2.0 * ws
    W11, W12 = Wp[:2048, :2048], Wp[:2048, 2048:]
    W21, W22 = Wp[2048:, :2048], Wp[2048:, 2048:]
    S1 = W21 + W22
    S2 = S1 - W11
    S3 = W11 - W21
    S4 = W12 - S2
    Ls = [W11, -W12, S4, -S1, S2, S3, W22]
    # L[128r+mi, 256c+128i+ki] -> arr[r, p, c, ki, i, mi] -> [r, ki, p, c, i, mi]
    arr = np.stack(
        [L.reshape(16, 128, 8, 2, 128).transpose(0, 2, 4, 3, 1) for L in Ls],
        axis=1)  # [r, p, c, ki, i, mi]
    arr = arr.reshape(16, 7, 4, 2, 128, 2, 128).transpose(0, 2, 4, 1, 3, 5, 6)
    return np.ascontiguousarray(arr).astype(NP_FP8)  # [r, q, ki, p, cc, i, mi]


def prepare_consts(inputs):
    """Fold sign(w), BN, bias and the 0/1-activation rowsum correction.

    The device computes, per layer, a_dev = [mmA~ >= thr] where
    mmA~ = W~sign @ a_dev_prev over {0,1} activations. Negative BN scales
    (alpha <= 0) are handled exactly by tracking a per-neuron flip bit
    (a_true = 1 - a_dev) that folds into the *next* layer's weight signs:
    with s~ = s * (1-2*flip_in), mm_full = 2*(s~ @ a_dev) - rowsum(s~)
    holds for any flip pattern. Thresholds use integer snapping (mmA is
    always an integer), making the device comparison tie-free/exact.
    Layers 2+3 carry weights W' = 2*W (Strassen packing), so their
    thresholds are doubled (still integers -> still tie-free).
    """
    consts = {}
    flip_in = np.zeros(IND)  # input layer: a_dev = [x >= 0] = ste_sign, exact
    for i in (1, 2, 3, 4):
        w = np.asarray(inputs[f"w{i}"]).astype(np.float64)
        b = np.asarray(inputs[f"b{i}"]).astype(np.float64)
        g = np.asarray(inputs[f"g{i}"]).astype(np.float64)
        be = np.asarray(inputs[f"be{i}"]).astype(np.float64)
        m = np.asarray(inputs[f"m{i}"]).astype(np.float64)
        v = np.asarray(inputs[f"v{i}"]).astype(np.float64)
        ws = np.where(w >= 0, 1.0, -1.0) * (1.0 - 2.0 * flip_in)  # [fo, fi]
        rowsum = ws.sum(axis=1)                                   # [fo]
        alpha = g / np.sqrt(v + EPS)
        if i < 4:
            # BN(mm_full + b) >= 0 with mm_full = 2*mmA - rowsum:
            #   alpha > 0:  a_true = [mmA >= u],  u = (m-b-be/a+rowsum)/2
            #   alpha < 0:  a_true = [mmA <= u] = 1 - [mmA >= floor(u)+1]
            #   alpha == 0: BN = be, constant sign
            u = (m - b - be / alpha_safe(alpha) + rowsum) / 2.0
            pos = alpha > 0
            thr = np.where(pos, np.ceil(u), np.floor(u) + 1.0)
            zero = alpha == 0
            if zero.any():
                # constant: a_true = [be >= 0]; force a_dev accordingly
                thr = np.where(zero & (be >= 0), -1e30, thr)
                thr = np.where(zero & (be < 0), 1e30, thr)
                pos = pos | zero
            flip_in = (~pos).astype(np.float64)
            scale = 1.0 if i == 1 else 2.0  # Strassen layers carry 2x weights
            consts.setdefault("_thrs", []).append(
                (scale * thr).reshape(32, 128).T.astype(np.float32))
            if i == 1:
                consts["w1dr"] = _pack_w_dr(ws.T)
            else:
                consts[f"wS{i}"] = _pack_strassen(ws)
        else:
            # logits = mmA*(2*alpha) + ((b - m - rowsum)*alpha + be), pad to 16
            scale = 2.0 * alpha
            beta = (b - m - rowsum) * alpha + be
            c4 = np.zeros((16, 2), np.float32)
            c4[:10, 0] = scale.astype(np.float32)
            c4[:10, 1] = beta.astype(np.float32)
            consts["c4"] = c4
            ws_t_pad = np.zeros((HID, 16), np.float64)
            ws_t_pad[:, :10] = ws.T
            # w4dr[ki, c, ko, m] = ws_t_pad[256*c + 128*ko + ki, m]
            w4 = ws_t_pad.reshape(16, 2, 128, 16).transpose(2, 0, 1, 3)
            consts["w4dr"] = np.ascontiguousarray(w4).astype(NP_FP8)
    consts["thrs"] = np.ascontiguousarray(
        np.stack(consts.pop("_thrs"), axis=1))  # [128, 3, 32]
    return consts


def alpha_safe(a):
    return np.where(a == 0, 1.0, a)


_PROG_CACHE = {}


def _get_program(bc=BC):
    if bc not in _PROG_CACHE:
        _PROG_CACHE[bc] = build_program(bc)
    return _PROG_CACHE[bc]


def kernel(**inputs):
    global LAST_RESULTS
    x = np.asarray(inputs["x"], np.float32)
    assert x.shape == (B, 784)
    consts = prepare_consts(inputs)
    # act1 = [x >= 0] in {0,1} fp8 packed host-side straight into the
    # feature-major device layout a1h[p, c, i, n] = a[256c+128i+p, n].
    a1_full = (x[:, :IND].T >= 0).astype(NP_FP8)  # [768, B]
    a1_full = a1_full.reshape(3, 2, 128, B).transpose(2, 0, 1, 3)

    nc = _get_program(BC)
    in_maps = []
    for c in range(N_CORES):
        m = {"a1h": np.ascontiguousarray(a1_full[:, :, :, c * BC:(c + 1) * BC])}
        m.update(consts)
        in_maps.append(m)

    res = run_bass_kernel_spmd(
        nc, in_maps, core_ids=list(range(N_CORES)), trace=TRACE,
        **TRACE_KWARGS)
    LAST_RESULTS = res
    # device out is [128, BT, 10] partition-major; restore batch order
    outs = [np.ascontiguousarray(r["out"].transpose(1, 0, 2).reshape(BC, OUT))
            for r in res.results]
    return np.concatenate(outs, axis=0)


# revision 21
# speedup vs baseline: 1.2513x; 1.0015x over previous
"""Binarized-MLP (BinaryNet) forward on 8 Trainium2 NeuronCores.

Reference computation (per nn_FC_large):
    h = sign(x[:, :768]) @ sign(w1).T + b1 ; BN1 ; -> sign
    h = sign(h) @ sign(w2).T + b2         ; BN2 ; -> sign
    h = sign(h) @ sign(w3).T + b3         ; BN3 ; -> sign
    h = sign(h) @ sign(w4).T + b4         ; BN4 ; log_softmax

Strategy (data parallel, batch 16384 -> 2048 rows/core):
  * Activations are a in {0,1} fp8 (a = [pre-act >= 0]); the identity
    sign_mm = 2*(Wsign @ a) - rowsum(Wsign) folds the rowsum into
    per-neuron integer thresholds, so every epilogue is one DVE op.
  * Matmuls run in fp8e4 with perf_mode=DoubleRow (K=256/instruction).
  * Layers 2+3 (4096x4096) use one level of Strassen-Winograd: 7 products
    of half-size blocks instead of 8 (PE work x7/8). Weight-side combos
    (host, values in {-8..8}) and activation-side combos T1..T4 (GpSimd,
    values in {-2..2}) are exact in fp8; products are integers well inside
    fp32, so the whole pipeline stays bit-exact w.r.t. the reference
    (weights carry a 2x so thresholds are simply doubled).
    The 7 output-side adds fold thresholds + compare into DVE
    scalar_tensor_tensor ops: e.g. C11 = [(P1 - t) >= (-P2)] with the
    (-P2) product negated host-side.
  * Layer-4 logits are PE-transposed to batch-major; log_softmax runs
    on-device (DVE/ACT) with Exp batched to avoid ACT table thrash.

Everything is hardcoded for x:[16384,784], layers 768->4096->4096->4096->10.
"""

import numpy as np
import ml_dtypes
from contextlib import ExitStack

import concourse.mybir as mybir
import concourse.tile as tile
from concourse import bacc
from concourse.bass_utils import run_bass_kernel_spmd
from concourse.masks import make_identity

FP32 = mybir.dt.float32
BF16 = mybir.dt.bfloat16
FP8 = mybir.dt.float8e4
NP_FP8 = ml_dtypes.float8_e4m3
NP_BF16 = ml_dtypes.bfloat16

EPS = 1e-5
B, IND, HID, OUT = 16384, 768, 4096, 10
N_CORES = 8
BC = B // N_CORES  # 2048 batch rows per core

# Knobs (test.py may flip TRACE before calling kernel()).
TRACE = False
TRACE_KWARGS = {}
LAST_RESULTS = None  # BassKernelResults of the most recent run

ALU = mybir.AluOpType


# --------------------------------------------------------------------------
# Device program
# --------------------------------------------------------------------------

def _emit_t12(nc, tpool, act_in, blk, lname):
    """t1 = B12-B11, t2 = B22-t1 on GpSimd (slow but emitted a whole block
    ahead into fresh pool slots, so it never gates the PE)."""
    nl = slice(512 * blk, 512 * blk + 512)
    nr = slice(1024 + 512 * blk, 1536 + 512 * blk)
    t1 = tpool.tile([128, 8, 2, 512], FP8, tag="t", bufs=5,
                    name=f"t1{lname}{blk}")
    t2 = tpool.tile([128, 8, 2, 512], FP8, tag="t", bufs=5,
                    name=f"t2{lname}{blk}")
    g = nc.gpsimd
    g.tensor_tensor(out=t1[:], in0=act_in[:, 0:8, :, nr],
                    in1=act_in[:, 0:8, :, nl], op=ALU.subtract)
    g.tensor_tensor(out=t2[:], in0=act_in[:, 8:16, :, nr], in1=t1[:],
                    op=ALU.subtract)
    return [t1, t2]


def _emit_t3(nc, tpool, act_in, blk, lname):
    """t3 = B22-B12 on DVE (fast; emitted mid-previous-block)."""
    nl = slice(512 * blk, 512 * blk + 512)
    nr = slice(1024 + 512 * blk, 1536 + 512 * blk)
    t3 = tpool.tile([128, 8, 2, 512], FP8, tag="t", bufs=5,
                    name=f"t3{lname}{blk}")
    nc.vector.tensor_tensor(out=t3[:], in0=act_in[:, 8:16, :, nr],
                            in1=act_in[:, 0:8, :, nr], op=ALU.subtract)
    return t3


def _emit_t4h(nc, tpool, act_in, blk, t2, half, lname):
    """One k-half of t4 = t2 - B21 on DVE (own small pool ring)."""
    nl = slice(512 * blk, 512 * blk + 512)
    cs = slice(4 * half, 4 * half + 4)
    t4 = tpool.tile([128, 4, 2, 512], FP8, tag="t4", bufs=3,
                    name=f"t4{lname}{blk}h{half}")
    nc.vector.tensor_tensor(out=t4[:], in0=t2[:, cs, :, :],
                            in1=act_in[:, 8 + 4 * half:12 + 4 * half, :, nl],
                            op=ALU.subtract)
    return t4


def _strassen_layer(nc, wpool, tpool, spool, psum_pool, act_in, wS, thr_sb,
                    act_out, bc, t_tiles, hooks=None):
    """One 4096x4096 binarized layer via 1-level Strassen-Winograd.

    act_in : SBUF AP [128, 16, 2, bc] fp8 {0,1}
    wS     : DRAM [16, 2, 128, 7, 4, 2, 128] fp8 (7 left-combos, DR layout)
    thr_sb : SBUF [128, 32] fp32 (2x integer-snapped thresholds)
    act_out: SBUF AP [128, 16, 2, bc] fp8
    t_tiles: [t-list for blk0, t-list for blk1] (emitted ahead)
    hooks  : dict (blk, r) -> callback() emitting a future block's T-combos
             at a point where its inputs are ready and DVE/GpSimd have slack.
    """
    DR = mybir.MatmulPerfMode.DoubleRow
    for blk in range(2):
        nl = slice(512 * blk, 512 * blk + 512)
        nr = slice(1024 + 512 * blk, 1536 + 512 * blk)
        t1, t2, t3, t4a, t4b = t_tiles[blk]  # filled ahead by hooks
        # rhs for products [P1, P2n, P3, P5n, P6, P7, P4]
        rhs = [lambda c: act_in[:, c, :, nl],
               lambda c: act_in[:, 8 + c, :, nl],
               lambda c: act_in[:, 8 + c, :, nr],
               lambda c: t1[:, c, :, :],
               lambda c: t2[:, c, :, :],
               lambda c: t3[:, c, :, :],
               lambda c: (t4a if c < 4 else t4b)[:, c % 4, :, :]]
        for r in range(16):
            t_top = thr_sb[:, r:r + 1]
            t_bot = thr_sb[:, 16 + r:17 + r]
            o_tl = act_out[:, r // 2, r % 2, nl]       # C11
            o_tr = act_out[:, r // 2, r % 2, nr]       # C12
            o_bl = act_out[:, 8 + r // 2, r % 2, nl]   # C21
            o_br = act_out[:, 8 + r // 2, r % 2, nr]   # C22
            pb = [psum_pool.tile([128, 512], FP32, tag="psum",
                                 name=f"sp{blk}_{r}_{p}") for p in range(7)]
            s0 = spool.tile([128, 512], FP32, tag="v", name=f"s0_{blk}{r}")
            v2 = spool.tile([128, 512], FP32, tag="v", name=f"v2_{blk}{r}")
            v3 = spool.tile([128, 512], FP32, tag="v", name=f"v3_{blk}{r}")
            v4 = spool.tile([128, 512], FP32, tag="v", name=f"v4_{blk}{r}")
            for q in range(4):
                wt = wpool.tile([128, 7, 2, 2, 128], FP8, tag="ws")
                nc.sync.dma_start(out=wt[:], in_=wS[r, q])
                for p in range(7):
                    for cc in range(2):
                        c = 2 * q + cc
                        nc.tensor.matmul(
                            pb[p][:], lhsT=wt[:, p, cc, :, :], rhs=rhs[p](c),
                            start=(c == 0), stop=(c == 7), perf_mode=DR)
            # DVE reads at most one PSUM operand (the STT in1 port), so
            # P1 is copied to SBUF once and every combine chains off SBUF.
            nc.vector.tensor_scalar(               # s0 = P1
                out=s0[:], in0=pb[0][:], scalar1=0.0, scalar2=None,
                op0=ALU.add)
            nc.vector.scalar_tensor_tensor(        # C11 = [(P1 - t) >= -P2]
                out=o_tl, in0=s0[:], scalar=t_top, in1=pb[1][:],
                op0=ALU.subtract, op1=ALU.is_ge)
            nc.vector.scalar_tensor_tensor(        # V2 = U2 = P1 + P6
                out=v2[:], in0=s0[:], scalar=0.0, in1=pb[4][:],
                op0=ALU.bypass, op1=ALU.add)
            nc.vector.scalar_tensor_tensor(        # V3 = U2 + P7 - t_bot
                out=v3[:], in0=v2[:], scalar=t_bot, in1=pb[5][:],
                op0=ALU.subtract, op1=ALU.add)
            nc.vector.scalar_tensor_tensor(        # V4 = U2 + P3 - t_top
                out=v4[:], in0=v2[:], scalar=t_top, in1=pb[2][:],
                op0=ALU.subtract, op1=ALU.add)
            nc.vector.scalar_tensor_tensor(        # C12 = [V4 >= -P5]
                out=o_tr, in0=v4[:], scalar=0.0, in1=pb[3][:],
                op0=ALU.bypass, op1=ALU.is_ge)
            nc.vector.scalar_tensor_tensor(        # C21 = [V3 >= P4]
                out=o_bl, in0=v3[:], scalar=0.0, in1=pb[6][:],
                op0=ALU.bypass, op1=ALU.is_ge)
            nc.vector.scalar_tensor_tensor(        # C22 = [V3 >= -P5]
                out=o_br, in0=v3[:], scalar=0.0, in1=pb[3][:],
                op0=ALU.bypass, op1=ALU.is_ge)
            if hooks and (blk, r) in hooks:
                hooks[(blk, r)]()


def build_program(bc=BC):
    """Build the per-core Bass/Tile program (SPMD; identical on all cores)."""
    NT = bc // 512
    BT = bc // 128
    DR = mybir.MatmulPerfMode.DoubleRow

    nc = bacc.Bacc(None, target_bir_lowering=False, debug=False)

    a1h = nc.dram_tensor("a1h", [128, 3, 2, bc], FP8, kind="ExternalInput")
    w1 = nc.dram_tensor("w1dr", [32, 128, 3, 2, 128], FP8, kind="ExternalInput")
    wS2 = nc.dram_tensor("wS2", [16, 4, 128, 7, 2, 2, 128], FP8,
                         kind="ExternalInput")
    wS3 = nc.dram_tensor("wS3", [16, 4, 128, 7, 2, 2, 128], FP8,
                         kind="ExternalInput")
    w4 = nc.dram_tensor("w4dr", [128, 16, 2, 16], FP8, kind="ExternalInput")
    thrs = nc.dram_tensor("thrs", [128, 3, 32], FP32, kind="ExternalInput")
    thrA = nc.dram_tensor("thrA", [128, 16, 2], FP32, kind="ExternalInput")
    c4 = nc.dram_tensor("c4", [16, 2], FP32, kind="ExternalInput")
    out = nc.dram_tensor("out", [128, bc // 128, OUT], FP32,
                         kind="ExternalOutput")

    with tile.TileContext(nc) as tc, ExitStack() as ctx:
        consts = ctx.enter_context(tc.tile_pool(name="consts", bufs=1))
        xpool = ctx.enter_context(tc.tile_pool(name="xpool", bufs=2))
        apool = ctx.enter_context(tc.tile_pool(name="apool", bufs=2))
        wpool = ctx.enter_context(tc.tile_pool(name="wpool", bufs=2))
        wspool = ctx.enter_context(tc.tile_pool(name="wspool", bufs=3))
        tpool = ctx.enter_context(tc.tile_pool(name="tpool", bufs=4))
        spool = ctx.enter_context(tc.tile_pool(name="spool", bufs=3))
        smpool = ctx.enter_context(tc.tile_pool(name="smpool", bufs=3))
        psum_pool = ctx.enter_context(
            tc.tile_pool(name="psum", bufs=8, space="PSUM"))

        thrs_sb = consts.tile([128, 3, 32], FP32, tag="thrs")
        thrA_sb = consts.tile([128, 16, 2], FP32, tag="thrA")
        c4_sb = consts.tile([16, 2], FP32, tag="c4")
        w4_sb = consts.tile([128, 16, 2, 16], FP8, tag="w4")
        ident = consts.tile([128, 128], FP32, tag="ident")
        identb = consts.tile([128, 128], BF16, tag="identb")
        h4 = consts.tile([16, bc], BF16, tag="h4")
        out_sb = consts.tile([128, BT, OUT], FP32, tag="outsb")
        thr1_sb = thrs_sb[:, 0, :]
        thr2_sb = thrs_sb[:, 1, :]
        thr3_sb = thrs_sb[:, 2, :]

        # ---- act1 = [x >= 0] {0,1} fp8 packed host-side, DMA'd direct.
        # act1 lives in the big act ring (slot0); act3 reuses the slot later
        # (its first write is ordered after act1's last read by Tile deps).
        act1big = apool.tile([128, 16, 2, bc], FP8, tag="actbig",
                             name="act1")
        act1 = act1big[:, 0:3, :, :]
        nc.sync.dma_start(out=act1[:, 0, :, :], in_=a1h[:, 0, :, :])
        nc.scalar.dma_start(out=act1[:, 1:3, :, :], in_=a1h[:, 1:3, :, :])

        # consts follow x on the rings; their data is needed much later
        nc.scalar.dma_start(out=thrs_sb[:], in_=thrs[:])
        nc.scalar.dma_start(out=thrA_sb[:], in_=thrA[:])
        nc.scalar.dma_start(out=c4_sb[:], in_=c4[:])
        nc.scalar.dma_start(out=w4_sb[:], in_=w4[:])
        make_identity(nc, ident[:])
        nc.vector.tensor_scalar(out=identb[:], in0=ident[:], scalar1=0.0,
                                scalar2=None, op0=ALU.add)

        # PE warm-up: spans the act1/w1 DMA wait so layer 1 opens at speed.
        warm = consts.tile([128, 2, 128], FP8, tag="warm")
        nc.gpsimd.memset(warm[:], 0.0)
        wps = psum_pool.tile([128, 512], FP32, tag="psum", name="warmps")
        for _ in range(24):
            nc.tensor.matmul(
                wps[:, 0:128], lhsT=warm[:, :, 0:128], rhs=warm[:],
                start=True, stop=True, perf_mode=DR)

        # ---- layer 1 (768 -> 4096), classic DR, in TWO n-phases so that
        # the batch columns Strassen block 0 needs ({0:512, 1024:1536}) are
        # complete halfway through -> GpSimd builds L2's T-combos during
        # phase B and layer 2 opens with zero stall.
        act2 = apool.tile([128, 16, 2, bc], FP8, tag="actbig")
        t2_b0 = None
        for phase in range(2):
            for mt in range(32):
                wt = wpool.tile([128, 3, 2, 128], FP8, tag="w")
                nc.sync.dma_start(out=wt[:], in_=w1[mt])
                pss = [psum_pool.tile([128, 512], FP32, tag="psum",
                                      name=f"l1ps{phase}_{mt}_{n}")
                       for n in range(2)]
                for i, n in enumerate((phase, phase + 2)):
                    for c in range(3):
                        nc.tensor.matmul(
                            pss[i][:],
                            lhsT=wt[:, c, :, :],
                            rhs=act1[:, c, :, 512 * n:512 * (n + 1)],
                            start=(c == 0), stop=(c == 2), perf_mode=DR)
                for i, n in enumerate((phase, phase + 2)):
                    dst = act2[:, mt // 2, mt % 2, 512 * n:512 * (n + 1)]
                    if mt % 2 == 0:   # {0,1} epilogue on DVE
                        nc.vector.tensor_scalar(
                            out=dst, in0=pss[i][:],
                            scalar1=thr1_sb[:, mt:mt + 1],
                            scalar2=None, op0=ALU.is_ge)
                    else:             # +-1 epilogue on the idle ACT engine
                        nc.scalar.activation(
                            dst, pss[i][:],
                            mybir.ActivationFunctionType.Sign,
                            bias=thrA_sb[:, mt // 2, 1:2],
                            scale=thrA_sb[:, mt // 2, 0:1])
            if phase == 0:
                t2_b0 = _emit_t12(nc, tpool, act2, 0, "L2")
                t2_b0.append(_emit_t3(nc, tpool, act2, 0, "L2"))
                t2_b0.append(_emit_t4h(nc, tpool, act2, 0, t2_b0[1], 0, "L2"))
                t2_b0.append(_emit_t4h(nc, tpool, act2, 0, t2_b0[1], 1, "L2"))

        # ---- layers 2+3: Strassen-Winograd ----
        # T-combos are emitted AHEAD of use: the gpsimd pair (t1,t2) a whole
        # block early into fresh pool slots, the DVE pair (t3,t4) mid-way
        # through the previous block where DVE has slack.
        act3 = apool.tile([128, 16, 2, bc], FP8, tag="actbig")
        act4 = apool.tile([128, 16, 2, bc], FP8, tag="actbig")
        t2t = [t2_b0, None]
        t3t = [None, None]

        def grow(tl, idx, val):
            tl[idx] = val if tl[idx] is None else tl[idx] + val

        def mk_hooks(tl, idx, act, blk, nm):
            def late():
                grow(tl, idx, [_emit_t3(nc, tpool, act, blk, nm)])
                grow(tl, idx, [_emit_t4h(nc, tpool, act, blk,
                                         tl[idx][1], 0, nm)])
                grow(tl, idx, [_emit_t4h(nc, tpool, act, blk,
                                         tl[idx][1], 1, nm)])
            return {
                (8,): lambda: grow(tl, idx, _emit_t12(nc, tpool, act, blk, nm)),
                (15,): late,
            }

        def at(blk, h):
            return {(blk, r[0]): f for r, f in h.items()}

        hooks_l2 = {}
        hooks_l2.update(at(0, mk_hooks(t2t, 1, act2, 1, "L2")))
        hooks_l2.update(at(1, mk_hooks(t3t, 0, act3, 0, "L3")))
        hooks_l3 = at(0, mk_hooks(t3t, 1, act3, 1, "L3"))
        _strassen_layer(nc, wspool, tpool, spool, psum_pool, act2, wS2,
                        thr2_sb, act3, bc, t2t, hooks=hooks_l2)
        _strassen_layer(nc, wspool, tpool, spool, psum_pool, act3, wS3,
                        thr3_sb, act4, bc, t3t, hooks=hooks_l3)

        # ---- layer 4: logits (M padded 10->16), affine folds BN+rowsum.
        sh = smpool.tile([128, BT, OUT], FP32, tag="sh", bufs=1)
        se = smpool.tile([128, BT], FP32, tag="se", bufs=1)
        ls = smpool.tile([128, BT], FP32, tag="ls", bufs=1)

        def _l4_softmax_head(g):
            # transpose group g's batch tiles + max/shift on DVE; runs one
            # n-group behind the L4 matmuls so the PE never stalls on it
            for bt in range(4 * g, 4 * g + 4):
                tp = psum_pool.tile([128, OUT], BF16, tag="psum",
                                    name=f"tp{bt}")
                nc.tensor.transpose(
                    tp[:], h4[0:OUT, 128 * bt:128 * (bt + 1)],
                    identb[0:OUT, 0:OUT])
                mx = smpool.tile([128, 1], FP32, tag="mx", name=f"mx{bt}")
                nc.vector.reduce_max(mx[:], tp[:], axis=mybir.AxisListType.X)
                nc.vector.tensor_scalar(
                    out=sh[:, bt, :], in0=tp[:], scalar1=mx[:], scalar2=None,
                    op0=ALU.subtract)
        for n in range(NT):
            ps4 = psum_pool.tile([16, 512], FP32, tag="psum", name=f"ps4_{n}")
            for c in range(16):
                nc.tensor.matmul(
                    ps4[:],
                    lhsT=w4_sb[:, c, :, :],
                    rhs=act4[:, c, :, 512 * n:512 * (n + 1)],
                    start=(c == 0),
                    stop=(c == 15),
                    perf_mode=DR,
                )
            # affine on DVE: out = in*scale + bias
            nc.vector.tensor_scalar(
                out=h4[:, 512 * n:512 * (n + 1)], in0=ps4[:],
                scalar1=c4_sb[:, 0:1], scalar2=c4_sb[:, 1:2],
                op0=ALU.mult, op1=ALU.add,
            )
        for g in range(NT):
            _l4_softmax_head(g)
        ex = smpool.tile([128, BT, OUT], FP32, tag="ex", bufs=1)
        for bt in range(BT):  # all Exp together: one ACT table load
            nc.scalar.activation(
                ex[:, bt, :], sh[:, bt, :], mybir.ActivationFunctionType.Exp,
                accum_out=se[:, bt:bt + 1])
        nc.scalar.activation(  # single Ln over all batch tiles
            ls[:], se[:], mybir.ActivationFunctionType.Ln)
        for bt in range(BT):
            nc.vector.tensor_scalar(
                out=out_sb[:, bt, :], in0=sh[:, bt, :],
                scalar1=ls[:, bt:bt + 1], scalar2=None,
                op0=ALU.subtract)

        # out dram is [128, BT, OUT] (partition-major, fully contiguous DMA);
        # the host reassembles batch order with a free transpose.
        nc.sync.dma_start(out=out[:], in_=out_sb[:])

    nc.compile()
    return nc


# --------------------------------------------------------------------------
# Host-side preparation
# --------------------------------------------------------------------------

def _pack_w_dr(ws_t):
    """[Fin, Fout] {-1,+1} -> [Mt, 128, C, 2, 128] fp8 DoubleRow layout.

    wdr[mt, ki, c, ko, mi] = ws_t[256*c + 128*ko + ki, 128*mt + mi]
    """
    fin, fout = ws_t.shape
    C, Mt = fin // 256, fout // 128
    w = ws_t.reshape(C, 2, 128, Mt, 128).transpose(3, 2, 0, 1, 4)
    return np.ascontiguousarray(w).astype(NP_FP8)


def _pack_strassen(ws):
    """[4096 out, 4096 in] +-1 -> [16, 128, 7, 8, 2, 128] fp8: the 7
    Winograd left-operand combos of W' = 2*ws in DoubleRow layout,
    product order [P1, P2n, P3, P5n, P6, P7, P4] (P2, P5 negated so the
    device combines become single is_ge scalar_tensor_tensor ops).
    Caller passes the already-scaled weights Wp."""
    Wp = ws
    W11, W12 = Wp[:2048, :2048], Wp[:2048, 2048:]
    W21, W22 = Wp[2048:, :2048], Wp[2048:, 2048:]
    S1 = W21 + W22
    S2 = S1 - W11
    S3 = W11 - W21
    S4 = W12 - S2
    Ls = [W11, -W12, S4, -S1, S2, S3, W22]
    # L[128r+mi, 256c+128i+ki] -> arr[r, p, c, ki, i, mi] -> [r, ki, p, c, i, mi]
    arr = np.stack(
        [L.reshape(16, 128, 8, 2, 128).transpose(0, 2, 4, 3, 1) for L in Ls],
        axis=1)  # [r, p, c, ki, i, mi]
    arr = arr.reshape(16, 7, 4, 2, 128, 2, 128).transpose(0, 2, 4, 1, 3, 5, 6)
    return np.ascontiguousarray(arr).astype(NP_FP8)  # [r, q, ki, p, cc, i, mi]


def prepare_consts(inputs):
    """Fold sign(w), BN, bias and the 0/1-activation rowsum correction.

    The device computes, per layer, a_dev = [mmA~ >= thr] where
    mmA~ = W~sign @ a_dev_prev over {0,1} activations. Negative BN scales
    (alpha <= 0) are handled exactly by tracking a per-neuron flip bit
    (a_true = 1 - a_dev) that folds into the *next* layer's weight signs:
    with s~ = s * (1-2*flip_in), mm_full = 2*(s~ @ a_dev) - rowsum(s~)
    holds for any flip pattern. Thresholds use integer snapping (mmA is
    always an integer), making the device comparison tie-free/exact.
    Layers 2+3 carry weights W' = 2*W (Strassen packing), so their
    thresholds are doubled (still integers -> still tie-free).
    """
    consts = {}
    flip_in = np.zeros(IND)  # input layer: a_dev = [x >= 0] = ste_sign, exact
    for i in (1, 2, 3, 4):
        w = np.asarray(inputs[f"w{i}"]).astype(np.float64)
        b = np.asarray(inputs[f"b{i}"]).astype(np.float64)
        g = np.asarray(inputs[f"g{i}"]).astype(np.float64)
        be = np.asarray(inputs[f"be{i}"]).astype(np.float64)
        m = np.asarray(inputs[f"m{i}"]).astype(np.float64)
        v = np.asarray(inputs[f"v{i}"]).astype(np.float64)
        ws = np.where(w >= 0, 1.0, -1.0) * (1.0 - 2.0 * flip_in)  # [fo, fi]
        if i == 2:
            # layer-1 odd mt tiles emit +-1 (ACT Sign); even mts emit {0,1}.
            # Fold per-k-slice convention into W: x2 for {0,1} slices (with
            # rowsum correction), x1 for +-1 slices.
            colscale = np.where((np.arange(ws.shape[1]) // 128) % 2 == 1,
                                1.0, 2.0)
            rowsum = ws[:, colscale == 2.0].sum(axis=1)
        else:
            colscale = None
            rowsum = ws.sum(axis=1)                               # [fo]
        alpha = g / np.sqrt(v + EPS)
        if i < 4:
            # BN(mm_full + b) >= 0 with mm_full = 2*mmA - rowsum:
            #   alpha > 0:  a_true = [mmA >= u],  u = (m-b-be/a+rowsum)/2
            #   alpha < 0:  a_true = [mmA <= u] = 1 - [mmA >= floor(u)+1]
            #   alpha == 0: BN = be, constant sign
            u = (m - b - be / alpha_safe(alpha) + rowsum)
            if i != 2:
                u = u / 2.0
            pos = alpha > 0
            thr = np.where(pos, np.ceil(u), np.floor(u) + 1.0)
            zero = alpha == 0
            if zero.any():
                # constant: a_true = [be >= 0]; force a_dev accordingly
                thr = np.where(zero & (be >= 0), -1e30, thr)
                thr = np.where(zero & (be < 0), 1e30, thr)
                pos = pos | zero
            flip_in = (~pos).astype(np.float64)
            # stored scale: layer-2 thresholds compare the mixed-parity
            # mm directly; layer 3 carries 2x weights -> 2x thresholds
            scale = 2.0 if i == 3 else 1.0
            consts.setdefault("_thrs", []).append(
                (scale * thr).reshape(32, 128).T.astype(np.float32))
            if i == 1:
                consts["w1dr"] = _pack_w_dr(ws.T)
                # ACT Sign params for odd mts: sign(scl*psum + bias), exact
                # (+-1, ties impossible: bias is a half-integer)
                t2d = thr.reshape(32, 128)       # [mt, 128]
                p2d = pos.reshape(32, 128)
                z2d = (alpha == 0).reshape(32, 128)
                scl = np.where(p2d, 1.0, -1.0)
                bia = np.where(p2d, 0.5 - t2d, t2d - 0.5)
                scl = np.where(z2d, 0.0, scl)
                bia = np.where(z2d, np.where(t2d < 0, 0.5, -0.5), bia)
                thrA = np.stack([scl[1::2], bia[1::2]], axis=-1)  # [16,128,2]
                consts["thrA"] = np.ascontiguousarray(
                    thrA.transpose(1, 0, 2)).astype(np.float32)
            elif i == 2:
                consts["wS2"] = _pack_strassen(ws * colscale)
            else:
                consts["wS3"] = _pack_strassen(2.0 * ws)
        else:
            # logits = mmA*(2*alpha) + ((b - m - rowsum)*alpha + be), pad to 16
            scale = 2.0 * alpha
            beta = (b - m - rowsum) * alpha + be
            c4 = np.zeros((16, 2), np.float32)
            c4[:10, 0] = scale.astype(np.float32)
            c4[:10, 1] = beta.astype(np.float32)
            consts["c4"] = c4
            ws_t_pad = np.zeros((HID, 16), np.float64)
            ws_t_pad[:, :10] = ws.T
            # w4dr[ki, c, ko, m] = ws_t_pad[256*c + 128*ko + ki, m]
            w4 = ws_t_pad.reshape(16, 2, 128, 16).transpose(2, 0, 1, 3)
            consts["w4dr"] = np.ascontiguousarray(w4).astype(NP_FP8)
    consts["thrs"] = np.ascontiguousarray(
        np.stack(consts.pop("_thrs"), axis=1))  # [128, 3, 32]
    return consts


def alpha_safe(a):
    return np.where(a == 0, 1.0, a)


_PROG_CACHE = {}


def _get_program(bc=BC):
    if bc not in _PROG_CACHE:
        _PROG_CACHE[bc] = build_program(bc)
    return _PROG_CACHE[bc]


def kernel(**inputs):
    global LAST_RESULTS
    x = np.asarray(inputs["x"], np.float32)
    assert x.shape == (B, 784)
    consts = prepare_consts(inputs)
    # act1 = [x >= 0] in {0,1} fp8 packed host-side straight into the
    # feature-major device layout a1h[p, c, i, n] = a[256c+128i+p, n].
    a1_full = (x[:, :IND].T >= 0).astype(NP_FP8)  # [768, B]
    a1_full = a1_full.reshape(3, 2, 128, B).transpose(2, 0, 1, 3)

    nc = _get_program(BC)
    in_maps = []
    for c in range(N_CORES):
        m = {"a1h": np.ascontiguousarray(a1_full[:, :, :, c * BC:(c + 1) * BC])}
        m.update(consts)
        in_maps.append(m)

    res = run_bass_kernel_spmd(
        nc, in_maps, core_ids=list(range(N_CORES)), trace=TRACE,
        **TRACE_KWARGS)
    LAST_RESULTS = res
    # device out is [128, BT, 10] partition-major; restore batch order
    outs = [np.ascontiguousarray(r["out"].transpose(1, 0, 2).reshape(BC, OUT))
            for r in res.results]
    return np.concatenate(outs, axis=0)


# revision 22
# speedup vs baseline: 1.2864x; 1.0280x over previous
"""Binarized-MLP (BinaryNet) forward on 8 Trainium2 NeuronCores.

Reference computation (per nn_FC_large):
    h = sign(x[:, :768]) @ sign(w1).T + b1 ; BN1 ; -> sign
    h = sign(h) @ sign(w2).T + b2         ; BN2 ; -> sign
    h = sign(h) @ sign(w3).T + b3         ; BN3 ; -> sign
    h = sign(h) @ sign(w4).T + b4         ; BN4 ; log_softmax

Strategy (data parallel, batch 16384 -> 2048 rows/core):
  * All matmul operands are exactly representable in fp8: weights are
    binarized host-side to {-1,+1}; activations are kept as a in {0,1}
    (a = [pre-act >= 0]) and the identity
        sign_mm = 2*(Wsign @ a) - rowsum(Wsign)
    folds rowsum into per-neuron thresholds, so each layer's epilogue is a
    single DVE is_ge producing the next layer's {0,1} fp8 activations.
  * Matmuls run in fp8e4 with perf_mode=DoubleRow (K=256 per instruction),
    activations stored feature-major [F, B] in SBUF across the whole net.
  * BatchNorm (eval) + bias fold into thresholds (layers 1-3) / an affine
    (layer 4). Layer-4 logits are PE-transposed to batch-major and
    log_softmax runs on-device (DVE/ACT).
  * Accumulation is exact: products are in {-1,0,1}, sums are integers
    well inside fp32, so the binary pipeline is bit-exact w.r.t. the
    reference up to thresholds ties (probability ~0 with random BN stats).

Everything is hardcoded for x:[16384,784], layers 768->4096->4096->4096->10.
"""

import numpy as np
import ml_dtypes
from contextlib import ExitStack

import concourse.mybir as mybir
import concourse.tile as tile
from concourse import bacc
from concourse.bass_utils import run_bass_kernel_spmd
from concourse.masks import make_identity

FP32 = mybir.dt.float32
BF16 = mybir.dt.bfloat16
FP8 = mybir.dt.float8e4
NP_FP8 = ml_dtypes.float8_e4m3
NP_BF16 = ml_dtypes.bfloat16

EPS = 1e-5
B, IND, HID, OUT = 16384, 768, 4096, 10
N_CORES = 8
BC = B // N_CORES  # 2048 batch rows per core

# Knobs (test.py may flip TRACE before calling kernel()).
TRACE = False
TRACE_KWARGS = {}
LAST_RESULTS = None  # BassKernelResults of the most recent run


# --------------------------------------------------------------------------
# Device program
# --------------------------------------------------------------------------

def _layer_fwd(nc, wpool, psum_pool, act_in, C, wdr, thr_sb, act_out, Mt, bc,
               dma_engine=None):
    """One binarized layer: act_out = [W_fp8dr.T @ act_in >= thr] in {0,1} fp8.

    act_in : SBUF AP [128, C, 2, bc] fp8 ({0,1})
    wdr    : DRAM [Mt, 128, C, 2, 128] fp8 ({-1,+1})
    thr_sb : SBUF [128, Mt] fp32
    act_out: SBUF AP [128, Mt//2, 2, bc] fp8
    """
    NT = bc // 512
    DR = mybir.MatmulPerfMode.DoubleRow
    dma_engine = dma_engine or nc.sync
    for mt in range(Mt):
        wt = wpool.tile([128, C, 2, 128], FP8, tag="w")
        dma_engine.dma_start(out=wt[:], in_=wdr[mt])
        pss = [psum_pool.tile([128, 512], FP32, tag="psum", name=f"ps{mt}_{n}")
               for n in range(NT)]
        for c in range(C):
            for n in range(NT):
                nc.tensor.matmul(
                    pss[n][:],
                    lhsT=wt[:, c, :, :],
                    rhs=act_in[:, c, :, 512 * n:512 * (n + 1)],
                    start=(c == 0),
                    stop=(c == C - 1),
                    perf_mode=DR,
                )
        for n in range(NT):
            nc.vector.tensor_scalar(
                out=act_out[:, mt // 2, mt % 2, 512 * n:512 * (n + 1)],
                in0=pss[n][:],
                scalar1=thr_sb[:, mt:mt + 1],
                scalar2=None,
                op0=mybir.AluOpType.is_ge,
            )


def build_program(bc=BC, dump_acts=False):
    """Build the per-core Bass/Tile program (SPMD; identical on all cores)."""
    NT = bc // 512
    BT = bc // 128
    DR = mybir.MatmulPerfMode.DoubleRow

    nc = bacc.Bacc(None, target_bir_lowering=False, debug=False)
    dbg = {}
    if dump_acts:
        for nm in ("act1d", "act2d", "act3d", "act4d"):
            cdim = 3 if nm == "act1d" else 16
            dbg[nm] = nc.dram_tensor(
                nm, [128, cdim, 2, bc], FP8, kind="ExternalOutput")
        dbg["h4d"] = nc.dram_tensor("h4d", [16, bc], FP32, kind="ExternalOutput")

    a1h = nc.dram_tensor("a1h", [128, 3, 2, bc], FP8, kind="ExternalInput")
    w1 = nc.dram_tensor("w1dr", [32, 128, 3, 2, 128], FP8, kind="ExternalInput")
    w2 = nc.dram_tensor("w2dr", [32, 128, 16, 2, 128], FP8, kind="ExternalInput")
    w3 = nc.dram_tensor("w3dr", [32, 128, 16, 2, 128], FP8, kind="ExternalInput")
    w4 = nc.dram_tensor("w4dr", [128, 16, 2, 16], FP8, kind="ExternalInput")
    thrs = nc.dram_tensor("thrs", [128, 3, 32], FP32, kind="ExternalInput")
    c4 = nc.dram_tensor("c4", [16, 2], FP32, kind="ExternalInput")
    out = nc.dram_tensor("out", [128, bc // 128, OUT], FP32,
                         kind="ExternalOutput")

    with tile.TileContext(nc) as tc, ExitStack() as ctx:
        consts = ctx.enter_context(tc.tile_pool(name="consts", bufs=1))
        xpool = ctx.enter_context(tc.tile_pool(name="xpool", bufs=2))
        a1pool = ctx.enter_context(tc.tile_pool(name="a1pool", bufs=1))
        apool = ctx.enter_context(
            tc.tile_pool(name="apool", bufs=3 if dump_acts else 2))
        wpool = ctx.enter_context(tc.tile_pool(name="wpool", bufs=4))
        smpool = ctx.enter_context(tc.tile_pool(name="smpool", bufs=3))
        psum_pool = ctx.enter_context(
            tc.tile_pool(name="psum", bufs=8, space="PSUM"))

        thrs_sb = consts.tile([128, 3, 32], FP32, tag="thrs")
        c4_sb = consts.tile([16, 2], FP32, tag="c4")
        w4_sb = consts.tile([128, 16, 2, 16], FP8, tag="w4")
        ident = consts.tile([128, 128], FP32, tag="ident")
        h4 = consts.tile([16, bc], FP32, tag="h4")
        out_sb = consts.tile([128, BT, OUT], FP32, tag="outsb")
        thr1_sb = thrs_sb[:, 0, :]
        thr2_sb = thrs_sb[:, 1, :]
        thr3_sb = thrs_sb[:, 2, :]

        # ---- act1 = [x >= 0] {0,1} fp8 packed host-side, DMA'd direct.
        act1 = a1pool.tile([128, 3, 2, bc], FP8, tag="act1")
        nc.sync.dma_start(out=act1[:, 0, :, :], in_=a1h[:, 0, :, :])
        nc.scalar.dma_start(out=act1[:, 1:3, :, :], in_=a1h[:, 1:3, :, :])

        # consts follow x on the rings; their data is needed much later
        nc.scalar.dma_start(out=thrs_sb[:], in_=thrs[:])
        nc.scalar.dma_start(out=c4_sb[:], in_=c4[:])
        nc.scalar.dma_start(out=w4_sb[:], in_=w4[:])
        make_identity(nc, ident[:])

        # PE warm-up: the HAM clock gate needs sustained matmul activity to
        # lift the PE from 1.2 to 2.4 GHz. The PE would otherwise idle ~2us
        # waiting on the act1 DMA and start the real stream cold; a few
        # garbage DR matmuls (memset operands, never-read psum) span the wait.
        warm = consts.tile([128, 2, 512], FP8, tag="warm")
        nc.gpsimd.memset(warm[:], 0.0)
        wps = psum_pool.tile([128, 512], FP32, tag="psum", name="warmps")
        for _ in range(12):
            nc.tensor.matmul(
                wps[:], lhsT=warm[:, :, 0:128], rhs=warm[:],
                start=True, stop=True, perf_mode=DR)


        # ---- layers 1-3 ----
        act2 = apool.tile([128, 16, 2, bc], FP8, tag="actbig")
        _layer_fwd(nc, wpool, psum_pool, act1, 3, w1, thr1_sb, act2, 32, bc,
                   dma_engine=nc.scalar)
        act3 = apool.tile([128, 16, 2, bc], FP8, tag="actbig")
        _layer_fwd(nc, wpool, psum_pool, act2, 16, w2, thr2_sb, act3, 32, bc)
        act4 = apool.tile([128, 16, 2, bc], FP8, tag="actbig")
        _layer_fwd(nc, wpool, psum_pool, act3, 16, w3, thr3_sb, act4, 32, bc)

        # ---- layer 4: logits (M padded 10->16), affine folds BN+rowsum.
        # Softmax is phased to avoid ACT table thrash (Exp/Ln swaps).
        sh = smpool.tile([128, BT, OUT], FP32, tag="sh", bufs=1)
        se = smpool.tile([128, BT], FP32, tag="se", bufs=1)
        ls = smpool.tile([128, BT], FP32, tag="ls", bufs=1)

        def _l4_softmax_head(g):
            # transpose group g's batch tiles + max/shift on DVE; runs one
            # n-group behind the L4 matmuls so the PE never stalls on it
            for bt in range(4 * g, 4 * g + 4):
                tp = psum_pool.tile([128, OUT], FP32, tag="psum",
                                    name=f"tp{bt}")
                nc.tensor.transpose(
                    tp[:], h4[0:OUT, 128 * bt:128 * (bt + 1)],
                    ident[0:OUT, 0:OUT])
                mx = smpool.tile([128, 1], FP32, tag="mx", name=f"mx{bt}")
                nc.vector.reduce_max(mx[:], tp[:], axis=mybir.AxisListType.X)
                nc.vector.tensor_scalar(
                    out=sh[:, bt, :], in0=tp[:], scalar1=mx[:], scalar2=None,
                    op0=mybir.AluOpType.subtract)
        for n in range(NT):
            ps4 = psum_pool.tile([16, 512], FP32, tag="psum", name=f"ps4_{n}")
            for c in range(16):
                nc.tensor.matmul(
                    ps4[:],
                    lhsT=w4_sb[:, c, :, :],
                    rhs=act4[:, c, :, 512 * n:512 * (n + 1)],
                    start=(c == 0),
                    stop=(c == 15),
                    perf_mode=DR,
                )
            # affine on DVE: out = in*scale + bias
            nc.vector.tensor_scalar(
                out=h4[:, 512 * n:512 * (n + 1)], in0=ps4[:],
                scalar1=c4_sb[:, 0:1], scalar2=c4_sb[:, 1:2],
                op0=mybir.AluOpType.mult, op1=mybir.AluOpType.add,
            )
        for g in range(NT):
            _l4_softmax_head(g)
        ex = smpool.tile([128, BT, OUT], FP32, tag="ex", bufs=1)
        for bt in range(BT):  # all Exp together: one ACT table load
            nc.scalar.activation(
                ex[:, bt, :], sh[:, bt, :], mybir.ActivationFunctionType.Exp,
                accum_out=se[:, bt:bt + 1])
        nc.scalar.activation(  # single Ln over all batch tiles
            ls[:], se[:], mybir.ActivationFunctionType.Ln)
        for bt in range(BT):
            nc.vector.tensor_scalar(
                out=out_sb[:, bt, :], in0=sh[:, bt, :],
                scalar1=ls[:, bt:bt + 1], scalar2=None,
                op0=mybir.AluOpType.subtract)

        # out dram is [128, BT, OUT] (partition-major, fully contiguous DMA);
        # the host reassembles batch order with a free transpose.
        nc.sync.dma_start(out=out[:], in_=out_sb[:])

        if dump_acts:
            nc.sync.dma_start(out=dbg["act1d"][:], in_=act1[:])
            nc.sync.dma_start(out=dbg["act2d"][:], in_=act2[:])
            nc.sync.dma_start(out=dbg["act3d"][:], in_=act3[:])
            nc.sync.dma_start(out=dbg["act4d"][:], in_=act4[:])
            nc.sync.dma_start(out=dbg["h4d"][:], in_=h4[:])

    nc.compile()
    return nc


# --------------------------------------------------------------------------
# Host-side preparation
# --------------------------------------------------------------------------

def _pack_w_dr(ws_t):
    """[Fin, Fout] {-1,+1} -> [Mt, 128, C, 2, 128] fp8 DoubleRow layout.

    wdr[mt, ki, c, ko, mi] = ws_t[256*c + 128*ko + ki, 128*mt + mi]
    """
    fin, fout = ws_t.shape
    C, Mt = fin // 256, fout // 128
    w = ws_t.reshape(C, 2, 128, Mt, 128).transpose(3, 2, 0, 1, 4)
    return np.ascontiguousarray(w).astype(NP_FP8)


def prepare_consts(inputs):
    """Fold sign(w), BN, bias and the 0/1-activation rowsum correction.

    The device computes, per layer, a_dev = [mmA~ >= thr] where
    mmA~ = W~sign @ a_dev_prev over {0,1} activations. Negative BN scales
    (alpha <= 0) are handled exactly by tracking a per-neuron flip bit
    (a_true = 1 - a_dev) that folds into the *next* layer's weight signs:
    with s~ = s * (1-2*flip_in), mm_full = 2*(s~ @ a_dev) - rowsum(s~)
    holds for any flip pattern. Thresholds use integer snapping (mmA is
    always an integer), making the device comparison tie-free/exact.
    """
    consts = {}
    flip_in = np.zeros(IND)  # input layer: a_dev = [x >= 0] = ste_sign, exact
    for i in (1, 2, 3, 4):
        w = np.asarray(inputs[f"w{i}"]).astype(np.float64)
        b = np.asarray(inputs[f"b{i}"]).astype(np.float64)
        g = np.asarray(inputs[f"g{i}"]).astype(np.float64)
        be = np.asarray(inputs[f"be{i}"]).astype(np.float64)
        m = np.asarray(inputs[f"m{i}"]).astype(np.float64)
        v = np.asarray(inputs[f"v{i}"]).astype(np.float64)
        ws = np.where(w >= 0, 1.0, -1.0) * (1.0 - 2.0 * flip_in)  # [fo, fi]
        rowsum = ws.sum(axis=1)                                   # [fo]
        alpha = g / np.sqrt(v + EPS)
        if i < 4:
            # BN(mm_full + b) >= 0 with mm_full = 2*mmA - rowsum:
            #   alpha > 0:  a_true = [mmA >= u],  u = (m-b-be/a+rowsum)/2
            #   alpha < 0:  a_true = [mmA <= u] = 1 - [mmA >= floor(u)+1]
            #   alpha == 0: BN = be, constant sign
            u = (m - b - be / alpha_safe(alpha) + rowsum) / 2.0
            pos = alpha > 0
            thr = np.where(pos, np.ceil(u), np.floor(u) + 1.0)
            zero = alpha == 0
            if zero.any():
                # constant: a_true = [be >= 0]; force a_dev accordingly
                thr = np.where(zero & (be >= 0), -1e30, thr)
                thr = np.where(zero & (be < 0), 1e30, thr)
                pos = pos | zero
            flip_in = (~pos).astype(np.float64)
            consts.setdefault("_thrs", []).append(
                thr.reshape(32, 128).T.astype(np.float32))
            consts[f"w{i}dr"] = _pack_w_dr(ws.T)
        else:
            # logits = mmA*(2*alpha) + ((b - m - rowsum)*alpha + be), pad to 16
            scale = 2.0 * alpha
            beta = (b - m - rowsum) * alpha + be
            c4 = np.zeros((16, 2), np.float32)
            c4[:10, 0] = scale.astype(np.float32)
            c4[:10, 1] = beta.astype(np.float32)
            consts["c4"] = c4
            ws_t_pad = np.zeros((HID, 16), np.float64)
            ws_t_pad[:, :10] = ws.T
            # w4dr[ki, c, ko, m] = ws_t_pad[256*c + 128*ko + ki, m]
            w4 = ws_t_pad.reshape(16, 2, 128, 16).transpose(2, 0, 1, 3)
            consts["w4dr"] = np.ascontiguousarray(w4).astype(NP_FP8)
    consts["thrs"] = np.ascontiguousarray(
        np.stack(consts.pop("_thrs"), axis=1))  # [128, 3, 32]
    return consts


def alpha_safe(a):
    return np.where(a == 0, 1.0, a)


_PROG_CACHE = {}


def _get_program(bc=BC):
    if bc not in _PROG_CACHE:
        _PROG_CACHE[bc] = build_program(bc)
    return _PROG_CACHE[bc]


def kernel(**inputs):
    global LAST_RESULTS
    x = np.asarray(inputs["x"], np.float32)
    assert x.shape == (B, 784)
    consts = prepare_consts(inputs)
    a1_full = (x[:, :IND].T >= 0).astype(NP_FP8)  # [768, B]
    a1_full = a1_full.reshape(3, 2, 128, B).transpose(2, 0, 1, 3)

    nc = _get_program(BC)
    in_maps = []
    for c in range(N_CORES):
        m = {"a1h": np.ascontiguousarray(a1_full[:, :, :, c * BC:(c + 1) * BC])}
        m.update(consts)
        in_maps.append(m)

    res = run_bass_kernel_spmd(
        nc, in_maps, core_ids=list(range(N_CORES)), trace=TRACE,
        **TRACE_KWARGS)
    LAST_RESULTS = res
    # device out is [128, BT, 10] partition-major; restore batch order
    outs = [np.ascontiguousarray(r["out"].transpose(1, 0, 2).reshape(BC, OUT))
            for r in res.results]
    return np.concatenate(outs, axis=0)



# revision 23
# speedup vs baseline: 1.2909x; 1.0035x over previous
"""Binarized-MLP (BinaryNet) forward on 8 Trainium2 NeuronCores.

Reference computation (per nn_FC_large):
    h = sign(x[:, :768]) @ sign(w1).T + b1 ; BN1 ; -> sign
    h = sign(h) @ sign(w2).T + b2         ; BN2 ; -> sign
    h = sign(h) @ sign(w3).T + b3         ; BN3 ; -> sign
    h = sign(h) @ sign(w4).T + b4         ; BN4 ; log_softmax

Strategy (data parallel, batch 16384 -> 2048 rows/core):
  * All matmul operands are exactly representable in fp8: weights are
    binarized host-side to {-1,+1}; activations are kept as a in {0,1}
    (a = [pre-act >= 0]) and the identity
        sign_mm = 2*(Wsign @ a) - rowsum(Wsign)
    folds rowsum into per-neuron thresholds, so each layer's epilogue is a
    single DVE is_ge producing the next layer's {0,1} fp8 activations.
  * Matmuls run in fp8e4 with perf_mode=DoubleRow (K=256 per instruction),
    activations stored feature-major [F, B] in SBUF across the whole net.
  * BatchNorm (eval) + bias fold into thresholds (layers 1-3) / an affine
    (layer 4). Layer-4 logits are PE-transposed to batch-major and
    log_softmax runs on-device (DVE/ACT).
  * Accumulation is exact: products are in {-1,0,1}, sums are integers
    well inside fp32, so the binary pipeline is bit-exact w.r.t. the
    reference up to thresholds ties (probability ~0 with random BN stats).

Everything is hardcoded for x:[16384,784], layers 768->4096->4096->4096->10.
"""

import numpy as np
import ml_dtypes
from contextlib import ExitStack

import concourse.mybir as mybir
import concourse.tile as tile
from concourse import bacc
from concourse.bass_utils import run_bass_kernel_spmd
from concourse.masks import make_identity

FP32 = mybir.dt.float32
BF16 = mybir.dt.bfloat16
FP8 = mybir.dt.float8e4
NP_FP8 = ml_dtypes.float8_e4m3
NP_BF16 = ml_dtypes.bfloat16

EPS = 1e-5
B, IND, HID, OUT = 16384, 768, 4096, 10
N_CORES = 8
BC = B // N_CORES  # 2048 batch rows per core

# Knobs (test.py may flip TRACE before calling kernel()).
TRACE = False
TRACE_KWARGS = {}
LAST_RESULTS = None  # BassKernelResults of the most recent run


# --------------------------------------------------------------------------
# Device program
# --------------------------------------------------------------------------

def _layer_fwd(nc, wpool, psum_pool, act_in, C, wdr, thr_sb, act_out, Mt, bc,
               dma_engine=None):
    """One binarized layer: act_out = [W_fp8dr.T @ act_in >= thr] in {0,1} fp8.

    act_in : SBUF AP [128, C, 2, bc] fp8 ({0,1})
    wdr    : DRAM [Mt, 128, C, 2, 128] fp8 ({-1,+1})
    thr_sb : SBUF [128, Mt] fp32
    act_out: SBUF AP [128, Mt//2, 2, bc] fp8
    """
    NT = bc // 512
    DR = mybir.MatmulPerfMode.DoubleRow
    dma_engine = dma_engine or nc.sync
    for mt in range(Mt):
        wt = wpool.tile([128, C, 2, 128], FP8, tag="w")
        dma_engine.dma_start(out=wt[:], in_=wdr[mt])
        pss = [psum_pool.tile([128, 512], FP32, tag="psum", name=f"ps{mt}_{n}")
               for n in range(NT)]
        for c in range(C):
            for n in range(NT):
                nc.tensor.matmul(
                    pss[n][:],
                    lhsT=wt[:, c, :, :],
                    rhs=act_in[:, c, :, 512 * n:512 * (n + 1)],
                    start=(c == 0),
                    stop=(c == C - 1),
                    perf_mode=DR,
                )
        for n in range(NT):
            nc.vector.tensor_scalar(
                out=act_out[:, mt // 2, mt % 2, 512 * n:512 * (n + 1)],
                in0=pss[n][:],
                scalar1=thr_sb[:, mt:mt + 1],
                scalar2=None,
                op0=mybir.AluOpType.is_ge,
            )


def build_program(bc=BC, dump_acts=False):
    """Build the per-core Bass/Tile program (SPMD; identical on all cores)."""
    NT = bc // 512
    BT = bc // 128
    DR = mybir.MatmulPerfMode.DoubleRow

    nc = bacc.Bacc(None, target_bir_lowering=False, debug=False)
    dbg = {}
    if dump_acts:
        for nm in ("act1d", "act2d", "act3d", "act4d"):
            cdim = 3 if nm == "act1d" else 16
            dbg[nm] = nc.dram_tensor(
                nm, [128, cdim, 2, bc], FP8, kind="ExternalOutput")
        dbg["h4d"] = nc.dram_tensor("h4d", [16, bc], FP32, kind="ExternalOutput")

    a1h = nc.dram_tensor("a1h", [128, 3, 2, bc], FP8, kind="ExternalInput")
    w1 = nc.dram_tensor("w1dr", [32, 128, 3, 2, 128], FP8, kind="ExternalInput")
    w2 = nc.dram_tensor("w2dr", [32, 128, 16, 2, 128], FP8, kind="ExternalInput")
    w3 = nc.dram_tensor("w3dr", [32, 128, 16, 2, 128], FP8, kind="ExternalInput")
    w4 = nc.dram_tensor("w4dr", [128, 16, 2, 16], FP8, kind="ExternalInput")
    thrs = nc.dram_tensor("thrs", [128, 3, 32], FP32, kind="ExternalInput")
    thrA = nc.dram_tensor("thrA", [128, 16, 2], FP32, kind="ExternalInput")
    c4 = nc.dram_tensor("c4", [16, 2], FP32, kind="ExternalInput")
    out = nc.dram_tensor("out", [128, bc // 128, OUT], FP32,
                         kind="ExternalOutput")

    with tile.TileContext(nc) as tc, ExitStack() as ctx:
        consts = ctx.enter_context(tc.tile_pool(name="consts", bufs=1))
        xpool = ctx.enter_context(tc.tile_pool(name="xpool", bufs=2))
        a1pool = ctx.enter_context(tc.tile_pool(name="a1pool", bufs=1))
        apool = ctx.enter_context(
            tc.tile_pool(name="apool", bufs=3 if dump_acts else 2))
        wpool = ctx.enter_context(tc.tile_pool(name="wpool", bufs=4))
        smpool = ctx.enter_context(tc.tile_pool(name="smpool", bufs=3))
        psum_pool = ctx.enter_context(
            tc.tile_pool(name="psum", bufs=8, space="PSUM"))

        thrs_sb = consts.tile([128, 3, 32], FP32, tag="thrs")
        thrA_sb = consts.tile([128, 16, 2], FP32, tag="thrA")
        c4_sb = consts.tile([16, 2], FP32, tag="c4")
        w4_sb = consts.tile([128, 16, 2, 16], FP8, tag="w4")
        ident = consts.tile([128, 128], FP32, tag="ident")
        h4 = consts.tile([16, bc], FP32, tag="h4")
        out_sb = consts.tile([128, BT, OUT], FP32, tag="outsb")
        thr1_sb = thrs_sb[:, 0, :]
        thr2_sb = thrs_sb[:, 1, :]
        thr3_sb = thrs_sb[:, 2, :]

        # ---- act1 = [x >= 0] {0,1} fp8 packed host-side, DMA'd direct.
        act1 = a1pool.tile([128, 3, 2, bc], FP8, tag="act1")
        nc.sync.dma_start(out=act1[:, 0, :, :], in_=a1h[:, 0, :, :])
        nc.scalar.dma_start(out=act1[:, 1:3, :, :], in_=a1h[:, 1:3, :, :])

        # consts follow x on the rings; their data is needed much later
        nc.scalar.dma_start(out=thrs_sb[:], in_=thrs[:])
        nc.scalar.dma_start(out=thrA_sb[:], in_=thrA[:])
        nc.scalar.dma_start(out=c4_sb[:], in_=c4[:])
        nc.scalar.dma_start(out=w4_sb[:], in_=w4[:])
        make_identity(nc, ident[:])

        # PE warm-up: the HAM clock gate needs sustained matmul activity to
        # lift the PE from 1.2 to 2.4 GHz. The PE would otherwise idle ~2us
        # waiting on the act1 DMA and start the real stream cold; a few
        # garbage DR matmuls (memset operands, never-read psum) span the wait.
        warm = consts.tile([128, 2, 512], FP8, tag="warm")
        nc.gpsimd.memset(warm[:], 0.0)
        wps = psum_pool.tile([128, 512], FP32, tag="psum", name="warmps")
        for _ in range(12):
            nc.tensor.matmul(
                wps[:], lhsT=warm[:, :, 0:128], rhs=warm[:],
                start=True, stop=True, perf_mode=DR)


        # ---- layer 1: DVE is the per-mt bottleneck (4 epilogue ops vs
        # 12 matmuls), so odd mts emit +-1 via ACT Sign instead; the mixed
        # {0,1}/+-1 convention folds into layer-2's weight column scales.
        act2 = apool.tile([128, 16, 2, bc], FP8, tag="actbig")
        for mt in range(32):
            wt = wpool.tile([128, 3, 2, 128], FP8, tag="w")
            nc.sync.dma_start(out=wt[:], in_=w1[mt])
            pss = [psum_pool.tile([128, 512], FP32, tag="psum",
                                  name=f"l1ps{mt}_{n}") for n in range(NT)]
            for c in range(3):
                for n in range(NT):
                    nc.tensor.matmul(
                        pss[n][:], lhsT=wt[:, c, :, :],
                        rhs=act1[:, c, :, 512 * n:512 * (n + 1)],
                        start=(c == 0), stop=(c == 2), perf_mode=DR)
            for n in range(NT):
                dst = act2[:, mt // 2, mt % 2, 512 * n:512 * (n + 1)]
                if mt % 2 == 0:
                    nc.vector.tensor_scalar(
                        out=dst, in0=pss[n][:],
                        scalar1=thr1_sb[:, mt:mt + 1], scalar2=None,
                        op0=mybir.AluOpType.is_ge)
                else:
                    nc.scalar.activation(
                        dst, pss[n][:], mybir.ActivationFunctionType.Sign,
                        bias=thrA_sb[:, mt // 2, 1:2],
                        scale=thrA_sb[:, mt // 2, 0:1])
        act3 = apool.tile([128, 16, 2, bc], FP8, tag="actbig")
        _layer_fwd(nc, wpool, psum_pool, act2, 16, w2, thr2_sb, act3, 32, bc)
        # ---- layer 3 in TWO n-phases; after each phase its completed
        # batch columns feed layer 4 + softmax heads + Exp immediately, so
        # only half of the L4/softmax chain trails the last L3 matmul.
        sh = smpool.tile([128, BT, OUT], FP32, tag="sh", bufs=1)
        se = smpool.tile([128, BT], FP32, tag="se", bufs=1)
        ls = smpool.tile([128, BT], FP32, tag="ls", bufs=1)
        ex = smpool.tile([128, BT, OUT], FP32, tag="ex", bufs=1)
        act4 = apool.tile([128, 16, 2, bc], FP8, tag="actbig")

        def _l4_softmax_head(g):
            # transpose group g's batch tiles + max/shift on DVE
            for bt in range(4 * g, 4 * g + 4):
                tp = psum_pool.tile([128, OUT], FP32, tag="psum",
                                    name=f"tp{bt}")
                nc.tensor.transpose(
                    tp[:], h4[0:OUT, 128 * bt:128 * (bt + 1)],
                    ident[0:OUT, 0:OUT])
                mx = smpool.tile([128, 1], FP32, tag="mx", name=f"mx{bt}")
                nc.vector.reduce_max(mx[:], tp[:], axis=mybir.AxisListType.X)
                nc.vector.tensor_scalar(
                    out=sh[:, bt, :], in0=tp[:], scalar1=mx[:], scalar2=None,
                    op0=mybir.AluOpType.subtract)

        for phase in range(2):
            for mt in range(32):
                wt = wpool.tile([128, 16, 2, 128], FP8, tag="w")
                nc.sync.dma_start(out=wt[:], in_=w3[mt])
                pss = [psum_pool.tile([128, 512], FP32, tag="psum",
                                      name=f"l3ps{phase}_{mt}_{i}")
                       for i in range(2)]
                for c in range(16):
                    for i, n in enumerate((phase, phase + 2)):
                        nc.tensor.matmul(
                            pss[i][:], lhsT=wt[:, c, :, :],
                            rhs=act3[:, c, :, 512 * n:512 * (n + 1)],
                            start=(c == 0), stop=(c == 15), perf_mode=DR)
                for i, n in enumerate((phase, phase + 2)):
                    nc.vector.tensor_scalar(
                        out=act4[:, mt // 2, mt % 2, 512 * n:512 * (n + 1)],
                        in0=pss[i][:], scalar1=thr3_sb[:, mt:mt + 1],
                        scalar2=None, op0=mybir.AluOpType.is_ge)
            for g in (phase, phase + 2):
                ps4 = psum_pool.tile([16, 512], FP32, tag="psum",
                                     name=f"ps4_{g}")
                for c in range(16):
                    nc.tensor.matmul(
                        ps4[:],
                        lhsT=w4_sb[:, c, :, :],
                        rhs=act4[:, c, :, 512 * g:512 * (g + 1)],
                        start=(c == 0),
                        stop=(c == 15),
                        perf_mode=DR,
                    )
                # affine on DVE: out = in*scale + bias
                nc.vector.tensor_scalar(
                    out=h4[:, 512 * g:512 * (g + 1)], in0=ps4[:],
                    scalar1=c4_sb[:, 0:1], scalar2=c4_sb[:, 1:2],
                    op0=mybir.AluOpType.mult, op1=mybir.AluOpType.add,
                )
            for g in (phase, phase + 2):
                _l4_softmax_head(g)
            for g in (phase, phase + 2):
                for bt in range(4 * g, 4 * g + 4):
                    nc.scalar.activation(
                        ex[:, bt, :], sh[:, bt, :],
                        mybir.ActivationFunctionType.Exp,
                        accum_out=se[:, bt:bt + 1])
        nc.scalar.activation(  # single Ln over all batch tiles
            ls[:], se[:], mybir.ActivationFunctionType.Ln)
        for bt in range(BT):
            nc.vector.tensor_scalar(
                out=out_sb[:, bt, :], in0=sh[:, bt, :],
                scalar1=ls[:, bt:bt + 1], scalar2=None,
                op0=mybir.AluOpType.subtract)

        # out dram is [128, BT, OUT] (partition-major, fully contiguous DMA);
        # the host reassembles batch order with a free transpose.
        nc.sync.dma_start(out=out[:], in_=out_sb[:])

        if dump_acts:
            nc.sync.dma_start(out=dbg["act1d"][:], in_=act1[:])
            nc.sync.dma_start(out=dbg["act2d"][:], in_=act2[:])
            nc.sync.dma_start(out=dbg["act3d"][:], in_=act3[:])
            nc.sync.dma_start(out=dbg["act4d"][:], in_=act4[:])
            nc.sync.dma_start(out=dbg["h4d"][:], in_=h4[:])

    nc.compile()
    return nc


# --------------------------------------------------------------------------
# Host-side preparation
# --------------------------------------------------------------------------

def _pack_w_dr(ws_t):
    """[Fin, Fout] {-1,+1} -> [Mt, 128, C, 2, 128] fp8 DoubleRow layout.

    wdr[mt, ki, c, ko, mi] = ws_t[256*c + 128*ko + ki, 128*mt + mi]
    """
    fin, fout = ws_t.shape
    C, Mt = fin // 256, fout // 128
    w = ws_t.reshape(C, 2, 128, Mt, 128).transpose(3, 2, 0, 1, 4)
    return np.ascontiguousarray(w).astype(NP_FP8)


def prepare_consts(inputs):
    """Fold sign(w), BN, bias and the 0/1-activation rowsum correction.

    The device computes, per layer, a_dev = [mmA~ >= thr] where
    mmA~ = W~sign @ a_dev_prev over {0,1} activations. Negative BN scales
    (alpha <= 0) are handled exactly by tracking a per-neuron flip bit
    (a_true = 1 - a_dev) that folds into the *next* layer's weight signs:
    with s~ = s * (1-2*flip_in), mm_full = 2*(s~ @ a_dev) - rowsum(s~)
    holds for any flip pattern. Thresholds use integer snapping (mmA is
    always an integer), making the device comparison tie-free/exact.
    """
    consts = {}
    flip_in = np.zeros(IND)  # input layer: a_dev = [x >= 0] = ste_sign, exact
    for i in (1, 2, 3, 4):
        w = np.asarray(inputs[f"w{i}"]).astype(np.float64)
        b = np.asarray(inputs[f"b{i}"]).astype(np.float64)
        g = np.asarray(inputs[f"g{i}"]).astype(np.float64)
        be = np.asarray(inputs[f"be{i}"]).astype(np.float64)
        m = np.asarray(inputs[f"m{i}"]).astype(np.float64)
        v = np.asarray(inputs[f"v{i}"]).astype(np.float64)
        ws = np.where(w >= 0, 1.0, -1.0) * (1.0 - 2.0 * flip_in)  # [fo, fi]
        if i == 2:
            # layer-1 odd mts emit +-1 (ACT Sign), even mts {0,1}: fold the
            # per-k-slice convention into W2 (x2 for {0,1} slices, with the
            # rowsum correction restricted to them). Values {+-1,+-2}: exact.
            colscale = np.where((np.arange(ws.shape[1]) // 128) % 2 == 1,
                                1.0, 2.0)
            rowsum = ws[:, colscale == 2.0].sum(axis=1)
        else:
            colscale = None
            rowsum = ws.sum(axis=1)                               # [fo]
        alpha = g / np.sqrt(v + EPS)
        if i < 4:
            # BN(mm_full + b) >= 0 with mm_full = 2*mmA - rowsum:
            #   alpha > 0:  a_true = [mmA >= u],  u = (m-b-be/a+rowsum)/2
            #   alpha < 0:  a_true = [mmA <= u] = 1 - [mmA >= floor(u)+1]
            #   alpha == 0: BN = be, constant sign
            u = m - b - be / alpha_safe(alpha) + rowsum
            if i != 2:
                u = u / 2.0  # i==2 compares the mixed-parity mm directly
            pos = alpha > 0
            thr = np.where(pos, np.ceil(u), np.floor(u) + 1.0)
            zero = alpha == 0
            if zero.any():
                # constant: a_true = [be >= 0]; force a_dev accordingly
                thr = np.where(zero & (be >= 0), -1e30, thr)
                thr = np.where(zero & (be < 0), 1e30, thr)
                pos = pos | zero
            flip_in = (~pos).astype(np.float64)
            consts.setdefault("_thrs", []).append(
                thr.reshape(32, 128).T.astype(np.float32))
            consts[f"w{i}dr"] = _pack_w_dr(
                (ws * colscale).T if i == 2 else ws.T)
            if i == 1:
                # ACT Sign params for odd mts: sign(scl*psum + bias); bias is
                # a half-integer and psum an integer, so ties are impossible
                # and the output is exactly +-1.
                t2d = thr.reshape(32, 128)
                p2d = pos.reshape(32, 128)
                z2d = zero.reshape(32, 128)
                scl = np.where(p2d, 1.0, -1.0)
                bia = np.where(p2d, 0.5 - t2d, t2d - 0.5)
                scl = np.where(z2d, 0.0, scl)
                bia = np.where(z2d, np.where(t2d < 0, 0.5, -0.5), bia)
                thrA = np.stack([scl[1::2], bia[1::2]], axis=-1)
                consts["thrA"] = np.ascontiguousarray(
                    thrA.transpose(1, 0, 2)).astype(np.float32)
        else:
            # logits = mmA*(2*alpha) + ((b - m - rowsum)*alpha + be), pad to 16
            scale = 2.0 * alpha
            beta = (b - m - rowsum) * alpha + be
            c4 = np.zeros((16, 2), np.float32)
            c4[:10, 0] = scale.astype(np.float32)
            c4[:10, 1] = beta.astype(np.float32)
            consts["c4"] = c4
            ws_t_pad = np.zeros((HID, 16), np.float64)
            ws_t_pad[:, :10] = ws.T
            # w4dr[ki, c, ko, m] = ws_t_pad[256*c + 128*ko + ki, m]
            w4 = ws_t_pad.reshape(16, 2, 128, 16).transpose(2, 0, 1, 3)
            consts["w4dr"] = np.ascontiguousarray(w4).astype(NP_FP8)
    consts["thrs"] = np.ascontiguousarray(
        np.stack(consts.pop("_thrs"), axis=1))  # [128, 3, 32]
    return consts


def alpha_safe(a):
    return np.where(a == 0, 1.0, a)


_PROG_CACHE = {}


def _get_program(bc=BC):
    if bc not in _PROG_CACHE:
        _PROG_CACHE[bc] = build_program(bc)
    return _PROG_CACHE[bc]


def kernel(**inputs):
    global LAST_RESULTS
    x = np.asarray(inputs["x"], np.float32)
    assert x.shape == (B, 784)
    consts = prepare_consts(inputs)
    a1_full = (x[:, :IND].T >= 0).astype(NP_FP8)  # [768, B]
    a1_full = a1_full.reshape(3, 2, 128, B).transpose(2, 0, 1, 3)

    nc = _get_program(BC)
    in_maps = []
    for c in range(N_CORES):
        m = {"a1h": np.ascontiguousarray(a1_full[:, :, :, c * BC:(c + 1) * BC])}
        m.update(consts)
        in_maps.append(m)

    res = run_bass_kernel_spmd(
        nc, in_maps, core_ids=list(range(N_CORES)), trace=TRACE,
        **TRACE_KWARGS)
    LAST_RESULTS = res
    # device out is [128, BT, 10] partition-major; restore batch order
    outs = [np.ascontiguousarray(r["out"].transpose(1, 0, 2).reshape(BC, OUT))
            for r in res.results]
    return np.concatenate(outs, axis=0)

